# revision 1
# baseline (speedup 1.0000x reference)
# Bass/Tile TRN2 kernel for nn_BlqSSM (Mamba2-SSD-style block with depthwise
# 3x3 conv, non-causal linear attention, LayerNorm gate, out-projection).
#
# Sharding: data-parallel over batch — 8 batches on 8 NeuronCores, weights
# replicated, no collectives. Each core computes one full batch element.
#
# Per-core dataflow (L = H*W = 4096, channel-major = [C, L], L-major = [L, C]):
#   1. in_proj (PE, f32r): Z = W_in @ X, channel-major. xBC rows are written
#      straight into zero-padded [128, 66, 66] conv buffers (border-only
#      memset on GPSIMD); dt rows -> [8, L]. Emission order dt -> BC -> xv so
#      the conv of the BC group and the dA chain start early.
#   2. dt chain (ACT): dA = softplus(dt + bias) * exp(A_log); softplus is
#      composed as Ln(Exp(x)+1) (gen3 act tables have no native softplus).
#      dA is transposed to L-major via PE transposes.
#   3. depthwise conv (PE, f32r): 9 taps as diag-weight matmuls accumulating
#      in PSUM over shifted views of the padded buffer; SiLU+bias fused into
#      the PSUM->SBUF evacuation on ACT.
#   4. SSD (PE), interleaved with the conv of the xv groups: per 128-row
#      L-chunk, transpose B and V to L-major; V^T is scaled by dA during its
#      evacuation (DVE). KV [64, 256] accumulates over all 32 chunks in one
#      PSUM tile. Then y = C @ KV per chunk, with the D-skip term added by
#      transpose-accumulating V into the same PSUM tile.
#   5. LayerNorm (DVE bn_stats/bn_aggr + ACT rsqrt) + gating with z (computed
#      lazily from X by PE) + transpose of yz + out_proj (PE, f32r).
import os
import sys

for _p in ("/opt/trn_rl_repo", os.path.expanduser("~/.axon_site/_ro/trn_rl_repo")):
    if os.path.isdir(_p) and _p not in sys.path:
        sys.path.insert(0, _p)

import numpy as np

import concourse.bass as bass
import concourse.mybir as mybir
import concourse.tile as tile
from concourse import bacc
from concourse.bass_utils import run_bass_kernel_spmd

F32 = mybir.dt.float32
F32R = mybir.dt.float32r
AF = mybir.ActivationFunctionType
ALU = mybir.AluOpType

DIM = 256
DSTATE = 64
NHEADS = 8
HEAD_DIM = DIM // NHEADS          # 32
CONV_DIM = DIM + 2 * DSTATE       # 384
D_IN_PROJ = 2 * DIM + 2 * DSTATE + NHEADS  # 648
LN_EPS = 1e-5
B_, H_, W_ = 8, 64, 64
L = H_ * W_                       # 4096
NCORES = 8
NCH = 8                           # 512-wide column chunks of L
LCH = 32                          # 128-wide column chunks of L
PADW = H_ + 2                     # 66


def _build(flags, reps=1):
    generic_d = flags["generic_d"]
    has_lnw = flags["has_lnw"]
    has_lnb = flags["has_lnb"]

    nc = bacc.Bacc("TRN2", target_bir_lowering=False, debug=False, num_devices=NCORES)

    xd = nc.dram_tensor("xb", [DIM, L], F32R, kind="ExternalInput")
    w_xbcdt = nc.dram_tensor("w_xbcdt", [DIM, CONV_DIM + NHEADS], F32R, kind="ExternalInput")
    w_z = nc.dram_tensor("w_z", [DIM, DIM], F32R, kind="ExternalInput")
    w_out = nc.dram_tensor("w_out", [DIM, DIM], F32R, kind="ExternalInput")
    convdiag = nc.dram_tensor("convdiag", [128, 27, 128], F32R, kind="ExternalInput")
    identd = nc.dram_tensor("identd", [128, 128], F32R, kind="ExternalInput")
    dtbias = nc.dram_tensor("dtbias", [NHEADS, 1], F32, kind="ExternalInput")
    expa = nc.dram_tensor("expa", [NHEADS, 1], F32, kind="ExternalInput")
    convb = nc.dram_tensor("convb", [128, 3], F32, kind="ExternalInput")
    dexp = nc.dram_tensor("dexp", [128, 2, 128], F32R, kind="ExternalInput")
    lnwv = nc.dram_tensor("lnwv", [DIM], F32, kind="ExternalInput")
    lnbv = nc.dram_tensor("lnbv", [DIM], F32, kind="ExternalInput")
    outd = nc.dram_tensor("outb", [DIM, L], F32, kind="ExternalOutput")

    with tile.TileContext(nc) as tc:
        _emit(nc, tc, locals(), generic_d, has_lnw, has_lnb, reps)
    nc.compile()
    return nc


def _emit(nc, tc, t, generic_d, has_lnw, has_lnb, reps=1):
    from contextlib import ExitStack

    xd, w_xbcdt, w_z, w_out = t["xd"], t["w_xbcdt"], t["w_z"], t["w_out"]
    convdiag, identd, dtbias, expa = t["convdiag"], t["identd"], t["dtbias"], t["expa"]
    convb, dexp, lnwv, lnbv, outd = t["convb"], t["dexp"], t["lnwv"], t["lnbv"], t["outd"]

    with ExitStack() as ctx:
        const = ctx.enter_context(tc.tile_pool(name="const", bufs=1))
        big = ctx.enter_context(tc.tile_pool(name="big", bufs=1))

        # ---- weights + X, ordered so the first matmuls unblock earliest ----
        wxb = const.tile([128, 2, CONV_DIM + NHEADS], F32R)
        nc.sync.dma_start(wxb[:], w_xbcdt.ap().rearrange("(t k) m -> k t m", k=128))
        X = big.tile([128, 2, L], F32R)
        xr = xd.ap().rearrange("(t k) l -> k t l", k=128)
        for c in range(4):
            for k in range(2):
                eng = nc.sync if k == 0 else nc.scalar
                eng.dma_start(X[:, k, c * 1024:(c + 1) * 1024], xr[:, k, c * 1024:(c + 1) * 1024])
        ident = const.tile([128, 128], F32R)
        nc.sync.dma_start(ident[:], identd.ap())
        dtb_t = const.tile([128, NHEADS], F32)
        nc.sync.dma_start(dtb_t[:], dtbias.ap().rearrange("a b -> b a").partition_broadcast(128))
        ea_t = const.tile([128, NHEADS], F32)
        nc.sync.dma_start(ea_t[:], expa.ap().rearrange("a b -> b a").partition_broadcast(128))
        cb = const.tile([128, 3], F32)
        nc.sync.dma_start(cb[:], convb.ap())
        eps_t = const.tile([128, 1], F32)
        nc.vector.memset(eps_t[:], LN_EPS)
        wz = const.tile([128, 2, DIM], F32R)
        nc.scalar.dma_start(wz[:], w_z.ap().rearrange("(t k) m -> k t m", k=128))
        wo = const.tile([128, 2, DIM], F32R)
        nc.scalar.dma_start(wo[:], w_out.ap().rearrange("(t k) m -> k t m", k=128))
        if generic_d:
            dxp = const.tile([128, 2, 128], F32R)
            nc.sync.dma_start(dxp[:], dexp.ap())
        if has_lnw:
            lnw_bc = const.tile([128, DIM], F32)
            nc.sync.dma_start(lnw_bc[:], lnwv.ap().unsqueeze(0).partition_broadcast(128))
        if has_lnb:
            lnb_bc = const.tile([128, DIM], F32)
            nc.sync.dma_start(lnb_bc[:], lnbv.ap().unsqueeze(0).partition_broadcast(128))

        # Pre-load the activation table set that holds BOTH Exp and Ln so the
        # dt chain doesn't thrash table loads when the scheduler interleaves.
        from concourse.hw_specs import get_activation_tables
        _tabs = list(get_activation_tables(nc.m.arch).items())
        _set6 = next(i for i, (_nm, s) in enumerate(_tabs)
                     if AF.Exp in s and AF.Ln in s)
        nc.scalar.add_instruction(mybir.InstLoadActFuncSet(
            name=nc.get_next_instruction_name(), act_func_set_id=_set6))

        # ---- persistent activations ----
        pads = [big.tile([128, PADW, PADW], F32R, name=f"pad{g}") for g in range(3)]
        V0 = big.tile([128, L], F32R)
        V1 = big.tile([128, L], F32R)
        Vg = [V0, V1]
        BC = big.tile([128, L], F32R)
        dAlm = big.tile([128, LCH, NHEADS], F32)
        kv_sb = big.tile([128, DIM], F32R)  # KV parked at partitions 64:128

        for _rep in range(reps):
            # border-only zeroing of the conv pads (interior fully overwritten)
            for g in range(3):
                p = pads[g][:].bitcast(F32)
                nc.gpsimd.memset(p[:, 0:1, :], 0.0)
                nc.gpsimd.memset(p[:, PADW - 1:PADW, :], 0.0)
                nc.gpsimd.memset(p[:, 1:PADW - 1, 0:1], 0.0)
                nc.gpsimd.memset(p[:, 1:PADW - 1, PADW - 1:PADW], 0.0)

            with tc.tile_pool(name="pconv", bufs=2, space="PSUM") as pconv:
                # ---- Phase 1: dt + BC in_proj, conv(BC), dA chain ----
                with (
                    tc.tile_pool(name="p1a", bufs=2, space="PSUM") as p1a,
                    tc.tile_pool(name="pdt", bufs=1, space="PSUM") as pdt,
                    tc.tile_pool(name="pda", bufs=1, space="PSUM") as pda,
                    tc.tile_pool(name="dacm", bufs=1) as dacm_pool,
                    tc.tile_pool(name="cdgp", bufs=1) as cdgp,
                ):
                    cdg2 = cdgp.tile([128, 9, 128], F32R, tag="cdg")
                    nc.sync.dma_start(cdg2[:], convdiag.ap()[:, 18:27, :])
                    dtraw = dacm_pool.tile([NHEADS, L], F32)
                    for n in range(NCH):
                        ps = pdt.tile([NHEADS, 512], F32)
                        for k in range(2):
                            nc.tensor.matmul(ps[:], wxb[:, k, CONV_DIM:],
                                             X[:, k, n * 512:(n + 1) * 512],
                                             start=(k == 0), stop=(k == 1))
                        nc.scalar.copy(dtraw[:, n * 512:(n + 1) * 512], ps[:])

                    for n in range(NCH):
                        ps = p1a.tile([128, 512], F32)
                        for k in range(2):
                            nc.tensor.matmul(ps[:], wxb[:, k, 2 * 128:3 * 128],
                                             X[:, k, n * 512:(n + 1) * 512],
                                             start=(k == 0), stop=(k == 1))
                        r0 = n * 8
                        nc.vector.tensor_copy(pads[2][:, r0 + 1:r0 + 9, 1:1 + W_],
                                              ps[:].rearrange("p (r w) -> p r w", w=W_))
                    for i in range(LCH):
                        pt = pda.tile([128, NHEADS], F32)
                        nc.tensor.transpose(pt[:], dtraw[:, i * 128:(i + 1) * 128],
                                            ident[0:NHEADS, 0:NHEADS].bitcast(F32))
                        nc.vector.tensor_copy(dAlm[:, i, :], pt[:])
                    for n in range(NCH):
                        pc = pconv.tile([128, 512], F32, tag="pc")
                        r0 = n * 8
                        for ti in range(9):
                            dy, dx = ti // 3, ti % 3
                            nc.tensor.matmul(
                                pc[:].rearrange("p (r w) -> p r w", w=W_),
                                cdg2[:, ti, :],
                                pads[2][:, r0 + dy:r0 + dy + 8, dx:dx + W_],
                                start=(ti == 0), stop=(ti == 8))
                        nc.scalar.activation(BC[:, n * 512:(n + 1) * 512], pc[:],
                                             AF.Silu, bias=cb[:, 2:3])
                    # dA = Ln(Exp(dt + bias) + 1) * exp(A_log), fused in L-major
                    for q in range(4):
                        qs = slice(q * (LCH // 4), (q + 1) * (LCH // 4))
                        nq = LCH // 4
                        nc.vector.tensor_add(
                            dAlm[:, qs, :], dAlm[:, qs, :],
                            dtb_t[:].unsqueeze(1).to_broadcast([128, nq, NHEADS]))
                        nc.scalar.activation(dAlm[:, qs, :], dAlm[:, qs, :], AF.Exp)
                        nc.scalar.activation(dAlm[:, qs, :], dAlm[:, qs, :], AF.Ln, bias=1.0)
                        nc.vector.tensor_mul(
                            dAlm[:, qs, :], dAlm[:, qs, :],
                            ea_t[:].unsqueeze(1).to_broadcast([128, nq, NHEADS]))

                # ---- Phase 2: xv in_proj + conv + B/V^T + KV, one pipeline ----
                with (
                    tc.tile_pool(name="p1", bufs=2, space="PSUM") as p1,
                    tc.tile_pool(name="pbt", bufs=1, space="PSUM") as pbt,
                    tc.tile_pool(name="pvt", bufs=2, space="PSUM") as pvt,
                    tc.tile_pool(name="pkv", bufs=1, space="PSUM") as pkv,
                    tc.tile_pool(name="sbt", bufs=3) as sbt,
                    tc.tile_pool(name="cdgp2", bufs=2) as cdgp2,
                ):
                    kvp = pkv.tile([DSTATE, DIM], F32)
                    cdgv = []
                    for g in range(2):
                        c = cdgp2.tile([128, 9, 128], F32R, tag="cdg2")
                        nc.sync.dma_start(c[:], convdiag.ap()[:, g * 9:(g + 1) * 9, :])
                        cdgv.append(c)

                    def inproj_xv(n):
                        for g in range(2):
                            ps = p1.tile([128, 512], F32, tag="p1")
                            for k in range(2):
                                nc.tensor.matmul(ps[:], wxb[:, k, g * 128:(g + 1) * 128],
                                                 X[:, k, n * 512:(n + 1) * 512],
                                                 start=(k == 0), stop=(k == 1))
                            r0 = n * 8
                            dst = pads[g][:, r0 + 1:r0 + 9, 1:1 + W_]
                            src2 = ps[:].rearrange("p (r w) -> p r w", w=W_)
                            if g == 1:
                                nc.vector.tensor_copy(dst, src2)
                            else:
                                nc.scalar.copy(dst, src2)

                    def conv_ssd(n):
                        for g in range(2):
                            pc = pconv.tile([128, 512], F32, tag="pc")
                            r0 = n * 8
                            for ti in range(9):
                                dy, dx = ti // 3, ti % 3
                                nc.tensor.matmul(
                                    pc[:].rearrange("p (r w) -> p r w", w=W_),
                                    cdgv[g][:, ti, :],
                                    pads[g][:, r0 + dy:r0 + dy + 8, dx:dx + W_],
                                    start=(ti == 0), stop=(ti == 8))
                            nc.scalar.activation(Vg[g][:, n * 512:(n + 1) * 512], pc[:],
                                                 AF.Silu, bias=cb[:, g:g + 1])
                        for i in range(4 * n, 4 * n + 4):
                            sl = slice(i * 128, (i + 1) * 128)
                            ptb = pbt.tile([128, DSTATE], F32)
                            nc.tensor.transpose(ptb[:].bitcast(F32R), BC[0:DSTATE, sl],
                                                ident[0:DSTATE, 0:DSTATE])
                            bt = sbt.tile([128, DSTATE], F32R, tag="bt")
                            nc.vector.tensor_copy(bt[:], ptb[:])

                            ptv = pvt.tile([128, DIM], F32)
                            nc.tensor.transpose(ptv[:, 0:128].bitcast(F32R), V0[:, sl], ident[:])
                            nc.tensor.transpose(ptv[:, 128:256].bitcast(F32R), V1[:, sl], ident[:])
                            vst = sbt.tile([128, DIM], F32R, tag="vst")
                            nc.vector.tensor_mul(
                                vst[:].rearrange("p (h q) -> p h q", h=NHEADS),
                                ptv[:].rearrange("p (h q) -> p h q", h=NHEADS),
                                dAlm[:, i, :].unsqueeze(2).to_broadcast([128, NHEADS, HEAD_DIM]),
                            )
                            nc.tensor.matmul(kvp[:], bt[:], vst[:],
                                             start=(i == 0), stop=(i == LCH - 1))

                    for n in range(NCH + 1):
                        if n < NCH:
                            inproj_xv(n)
                        if n >= 1:
                            conv_ssd(n - 1)
                    nc.scalar.copy(kv_sb[DSTATE:2 * DSTATE, :], kvp[:])

            # ---- Phase 3: y, LN, gate, transpose, out_proj ----
            # Processed two L-chunks per iteration to amortize DVE/ACT per-op
            # overheads; y0,y1,z0,z1 share one 2-bank PSUM tile; the yz
            # transpose + out_proj trail one pair behind the LN chain.
            with (
                tc.tile_pool(name="pyp", bufs=3, space="PSUM") as pyp,
                tc.tile_pool(name="pzp", bufs=2, space="PSUM") as pzp,
                tc.tile_pool(name="pyzt", bufs=2, space="PSUM") as pyzt,
                tc.tile_pool(name="pout", bufs=1, space="PSUM") as pout,
                tc.tile_pool(name="s6", bufs=5) as s6,
                tc.tile_pool(name="s6g", bufs=2) as s6g,
            ):
                yzs = [None] * (LCH // 2)
                yzt_grp = None

                def emit_front(p):
                    ypr = pyp.tile([128, 2, DIM], F32, tag="ypr")
                    zpr = pzp.tile([128, 2, DIM], F32, tag="zpr")
                    for j in range(2):
                        i = 2 * p + j
                        sl = slice(i * 128, (i + 1) * 128)
                        for k in range(2):
                            nc.tensor.matmul(zpr[:, j, :], X[:, k, sl], wz[:, k, :],
                                             start=(k == 0), stop=(k == 1))
                    for j in range(2):
                        i = 2 * p + j
                        sl = slice(i * 128, (i + 1) * 128)
                        yp = ypr[:, j, :]
                        nc.tensor.matmul(yp, BC[DSTATE:2 * DSTATE, sl],
                                         kv_sb[DSTATE:2 * DSTATE, :], start=True, stop=False)
                        if generic_d:
                            for g in range(2):
                                nc.tensor.matmul(yp[:, g * 128:(g + 1) * 128], Vg[g][:, sl],
                                                 dxp[:, g, :], start=False, stop=True)
                        else:
                            for g in range(2):
                                nc.tensor.matmul(yp[:, g * 128:(g + 1) * 128].bitcast(F32R),
                                                 Vg[g][:, sl], ident[:],
                                                 is_transpose=True, start=False, stop=True)

                    st = s6.tile([128, 2, 6], F32, tag="st")
                    for j in range(2):
                        nc.vector.bn_stats(st[:, j, :], ypr[:, j, :])
                    mv = s6.tile([128, 2, 2], F32, tag="mv")
                    for j in range(2):
                        nc.vector.bn_aggr(mv[:, j, :], st[:, j, :])
                    rstd = s6.tile([128, 2], F32, tag="rstd")
                    nc.scalar.activation(rstd[:], mv[:, :, 1], AF.Abs_reciprocal_sqrt,
                                         bias=eps_t[:])
                    negm = s6.tile([128, 2], F32, tag="negm")
                    nc.vector.tensor_scalar_mul(negm[:], mv[:, :, 0], -1.0)
                    nmr = s6.tile([128, 2], F32, tag="nmr")
                    nc.vector.tensor_mul(nmr[:], negm[:], rstd[:])
                    yn = s6.tile([128, 2, DIM], F32, tag="yn")
                    for j in range(2):
                        nc.scalar.activation(yn[:, j, :], ypr[:, j, :], AF.Identity,
                                             bias=nmr[:, j:j + 1], scale=rstd[:, j:j + 1])
                    if has_lnw:
                        for j in range(2):
                            nc.vector.tensor_mul(yn[:, j, :], yn[:, j, :], lnw_bc[:])
                    if has_lnb:
                        for j in range(2):
                            nc.vector.tensor_add(yn[:, j, :], yn[:, j, :], lnb_bc[:])
                    yz = s6.tile([128, 2, DIM], F32R, tag="yz")
                    nc.vector.tensor_mul(yz[:], yn[:], zpr[:])
                    yzs[p] = yz

                def emit_back(p):
                    nonlocal yzt_grp
                    yz = yzs[p]
                    pt = pyzt.tile([128, 4, 128], F32)
                    for j in range(2):
                        for g in range(2):
                            nc.tensor.transpose(pt[:, 2 * j + g, :].bitcast(F32R),
                                                yz[:, j, g * 128:(g + 1) * 128], ident[:])
                    if p % 2 == 0:
                        yzt_grp = s6g.tile([128, 2, 512], F32R, tag="yzt")
                    # pt layout [j, g] -> dest [g, j]: one permuted-AP copy
                    nc.scalar.copy(
                        yzt_grp[:, :, (p % 2) * 256:(p % 2) * 256 + 256]
                        .rearrange("a g (j c) -> a g j c", j=2),
                        pt[:].rearrange("a (j g) c -> a g j c", j=2))
                    if p % 2 == 1:
                        n = p // 2
                        for mo in range(2):
                            po = pout.tile([128, 512], F32)
                            for k in range(2):
                                nc.tensor.matmul(po[:], wo[:, k, mo * 128:(mo + 1) * 128],
                                                 yzt_grp[:, k, :], start=(k == 0), stop=(k == 1))
                            ob = s6g.tile([128, 512], F32, tag="ob")
                            nc.scalar.copy(ob[:], po[:])
                            eng = nc.sync if (2 * n + mo) % 2 == 0 else nc.scalar
                            eng.dma_start(
                                outd.ap()[mo * 128:(mo + 1) * 128, n * 512:(n + 1) * 512],
                                ob[:])

                NP = LCH // 2
                PIPE = 3
                for p in range(NP + PIPE):
                    if p < NP:
                        emit_front(p)
                    if p >= PIPE:
                        emit_back(p - PIPE)


_CACHE = {}


def _prep(W_in, conv_w, conv_b, A_log, dt_bias, D, ln_w, ln_b, W_out):
    W_in = np.asarray(W_in, np.float32)
    conv_w = np.asarray(conv_w, np.float32)
    conv_b = np.asarray(conv_b, np.float32)
    A_log = np.asarray(A_log, np.float32)
    dt_bias = np.asarray(dt_bias, np.float32)
    D = np.asarray(D, np.float32)
    ln_w = np.asarray(ln_w, np.float32)
    ln_b = np.asarray(ln_b, np.float32)
    W_out = np.asarray(W_out, np.float32)

    WinT = np.ascontiguousarray(W_in.T)                       # [256, 648]
    w_xbcdt = np.ascontiguousarray(WinT[:, DIM:])             # [256, 392]
    w_z = np.ascontiguousarray(WinT[:, :DIM])                 # [256, 256]
    w_out_t = np.ascontiguousarray(W_out.T)                   # [256, 256]

    cd = np.zeros((128, 27, 128), np.float32)
    idx = np.arange(128)
    for g in range(3):
        for t in range(9):
            cd[idx, g * 9 + t, idx] = conv_w[g * 128 + idx, 0, t // 3, t % 3]

    ident = np.eye(128, dtype=np.float32)
    dexp = np.zeros((128, 2, 128), np.float32)
    d_expand = np.repeat(D, HEAD_DIM)                          # [256]
    for g in range(2):
        dexp[idx, g, idx] = d_expand[g * 128 + idx]

    cbm = np.zeros((128, 3), np.float32)
    for g in range(3):
        cbm[:, g] = conv_b[g * 128:(g + 1) * 128]

    flags = {
        "generic_d": not np.allclose(D, 1.0),
        "has_lnw": not np.allclose(ln_w, 1.0),
        "has_lnb": not np.allclose(ln_b, 0.0),
    }
    shared = dict(
        w_xbcdt=w_xbcdt, w_z=w_z, w_out=w_out_t, convdiag=cd, identd=ident,
        dtbias=dt_bias.reshape(NHEADS, 1).astype(np.float32),
        expa=np.exp(A_log).reshape(NHEADS, 1).astype(np.float32),
        convb=cbm, dexp=dexp, lnwv=ln_w, lnbv=ln_b,
    )
    return flags, shared


def _get_nc(flags, reps=1):
    key = (tuple(sorted(flags.items())), reps)
    if key not in _CACHE:
        _CACHE[key] = _build(flags, reps)
    return _CACHE[key]


def kernel(x, W_in, conv_w, conv_b, A_log, dt_bias, D, ln_w, ln_b, W_out,
           _trace=False):
    x = np.asarray(x, np.float32)
    flags, shared = _prep(W_in, conv_w, conv_b, A_log, dt_bias, D, ln_w, ln_b, W_out)
    nc = _get_nc(flags)
    xb = x.reshape(B_, DIM, L)
    in_maps = [dict(xb=np.ascontiguousarray(xb[b]), **shared) for b in range(B_)]
    res = run_bass_kernel_spmd(nc, in_maps, core_ids=list(range(NCORES)), trace=_trace)
    out = np.stack([res.results[b]["outb"] for b in range(B_)])
    out = out.reshape(B_, DIM, H_, W_)
    if _trace:
        return out, res
    return out



# revision 4
# speedup vs baseline: 1.0332x; 1.0332x over previous
# Bass/Tile TRN2 kernel for nn_BlqSSM (Mamba2-SSD-style block with depthwise
# 3x3 conv, non-causal linear attention, LayerNorm gate, out-projection).
#
# Sharding: data-parallel over batch — 8 batches on 8 NeuronCores, weights
# replicated, no collectives. Each core computes one full batch element.
#
# v2: bf16 datapath + XBAR DMA transposes. All PE-transpose matmuls from v1
# (V^T, B^T, dA^T, yz^T, D-skip) are replaced by dma_start_transpose (bf16
# only) or DVE adds, and activations/weights stream through the PE in bf16
# (1 cycle/row, half-size LDWEIGHTS). PSUM accumulation stays fp32.
#
# Per-core dataflow (L = H*W = 4096, channel-major = [C, L], L-major = [L, C]):
#   1. in_proj (PE, bf16): dt rows -> [16, L] (top 8 zero) -> XBAR transpose;
#      dA = Ln(Exp(dt + bias) + 1) * exp(A_log) in one contiguous ACT block
#      (exactly 3 act-table loads in the whole kernel). BC rows are written
#      into a zero-padded [128, 66, 66] conv buffer; z is computed
#      channel-major with stationary W_z slices and XBAR-transposed to
#      L-major during phase 2.
#   2. depthwise conv (PE, bf16): 9 taps as diag-weight matmuls accumulating
#      in PSUM over shifted views of the padded buffer; SiLU+bias fused into
#      the PSUM->SBUF evacuation on ACT, emitting bf16.
#   3. SSD (PE): per 512-col chunk, XBAR-transpose B and V to L-major;
#      V^T is scaled by dA on DVE (bf16); KV [64, 256] accumulates over all
#      32 chunks in one PSUM tile.
#   4. y = C @ KV per 128-row chunk; the D-skip term (y += V*D) is a DVE add
#      of the saved V^T directly into PSUM. LayerNorm (DVE bn_stats/bn_aggr
#      + ACT rsqrt) + gating with z; yz (bf16) is XBAR-transposed back to
#      channel-major and fed to out_proj (PE, bf16), output DMA'd as fp32.
import os
import sys

for _p in ("/opt/trn_rl_repo", os.path.expanduser("~/.axon_site/_ro/trn_rl_repo")):
    if os.path.isdir(_p) and _p not in sys.path:
        sys.path.insert(0, _p)

import numpy as np
import ml_dtypes

import concourse.bass as bass
import concourse.mybir as mybir
import concourse.tile as tile
from concourse import bacc
from concourse.bass_utils import run_bass_kernel_spmd

F32 = mybir.dt.float32
BF16 = mybir.dt.bfloat16
AF = mybir.ActivationFunctionType
ALU = mybir.AluOpType

DIM = 256
DSTATE = 64
NHEADS = 8
HEAD_DIM = DIM // NHEADS          # 32
CONV_DIM = DIM + 2 * DSTATE       # 384
D_IN_PROJ = 2 * DIM + 2 * DSTATE + NHEADS  # 648
LN_EPS = 1e-5
B_, H_, W_ = 8, 64, 64
L = H_ * W_                       # 4096
NCORES = 8
NCH = 8                           # 512-wide column chunks of L
LCH = 32                          # 128-wide column chunks of L
PADW = H_ + 2                     # 66

BF = ml_dtypes.bfloat16


def _build(flags, reps=1):
    generic_d = flags["generic_d"]
    has_lnw = flags["has_lnw"]
    has_lnb = flags["has_lnb"]

    nc = bacc.Bacc("TRN2", target_bir_lowering=False, debug=False, num_devices=NCORES)

    xd = nc.dram_tensor("xb", [DIM, L], BF16, kind="ExternalInput")
    w_xbcdt = nc.dram_tensor("w_xbcdt", [DIM, CONV_DIM + NHEADS], BF16, kind="ExternalInput")
    w_z = nc.dram_tensor("w_z", [DIM, DIM], BF16, kind="ExternalInput")
    w_out = nc.dram_tensor("w_out", [DIM, DIM], BF16, kind="ExternalInput")
    convdiag = nc.dram_tensor("convdiag", [128, 27, 128], BF16, kind="ExternalInput")
    dtbias = nc.dram_tensor("dtbias", [NHEADS, 1], F32, kind="ExternalInput")
    expa = nc.dram_tensor("expa", [NHEADS, 1], F32, kind="ExternalInput")
    convb = nc.dram_tensor("convb", [128, 3], F32, kind="ExternalInput")
    dexpand = nc.dram_tensor("dexpand", [1, DIM], F32, kind="ExternalInput")
    lnwv = nc.dram_tensor("lnwv", [DIM], F32, kind="ExternalInput")
    lnbv = nc.dram_tensor("lnbv", [DIM], F32, kind="ExternalInput")
    outd = nc.dram_tensor("outb", [DIM, L], F32, kind="ExternalOutput")

    with tile.TileContext(nc) as tc:
        _emit(nc, tc, locals(), generic_d, has_lnw, has_lnb, reps)
    nc.compile()
    return nc


def _emit(nc, tc, t, generic_d, has_lnw, has_lnb, reps=1):
    from contextlib import ExitStack

    xd, w_xbcdt, w_z, w_out = t["xd"], t["w_xbcdt"], t["w_z"], t["w_out"]
    convdiag, dtbias, expa = t["convdiag"], t["dtbias"], t["expa"]
    convb, dexpand, lnwv, lnbv, outd = t["convb"], t["dexpand"], t["lnwv"], t["lnbv"], t["outd"]

    with ExitStack() as ctx:
        const = ctx.enter_context(tc.tile_pool(name="const", bufs=1))
        big = ctx.enter_context(tc.tile_pool(name="big", bufs=1))

        # ---- weights + X, ordered so the first matmuls unblock earliest ----
        wxb = const.tile([128, 2, CONV_DIM + NHEADS], BF16)
        nc.sync.dma_start(wxb[:], w_xbcdt.ap().rearrange("(t k) m -> k t m", k=128))
        X = big.tile([128, 2, L], BF16)
        xr = xd.ap().rearrange("(t k) l -> k t l", k=128)
        for c in range(4):
            for k in range(2):
                eng = nc.sync if k == 0 else nc.scalar
                eng.dma_start(X[:, k, c * 1024:(c + 1) * 1024], xr[:, k, c * 1024:(c + 1) * 1024])
        dtb_t = const.tile([128, NHEADS], F32)
        nc.sync.dma_start(dtb_t[:], dtbias.ap().rearrange("a b -> b a").partition_broadcast(128))
        ea_t = const.tile([128, NHEADS], F32)
        nc.sync.dma_start(ea_t[:], expa.ap().rearrange("a b -> b a").partition_broadcast(128))
        cb = const.tile([128, 3], F32)
        nc.sync.dma_start(cb[:], convb.ap())
        eps_t = const.tile([128, 1], F32)
        nc.vector.memset(eps_t[:], LN_EPS)
        wz = const.tile([128, 2, DIM], BF16)
        nc.scalar.dma_start(wz[:], w_z.ap().rearrange("(t k) m -> k t m", k=128))
        wo = const.tile([128, 2, DIM], BF16)
        nc.scalar.dma_start(wo[:], w_out.ap().rearrange("(t k) m -> k t m", k=128))
        if generic_d:
            dex_bc = const.tile([128, DIM], F32)
            nc.sync.dma_start(dex_bc[:], dexpand.ap().partition_broadcast(128))
        if has_lnw:
            lnw_bc = const.tile([128, DIM], F32)
            nc.sync.dma_start(lnw_bc[:], lnwv.ap().unsqueeze(0).partition_broadcast(128))
        if has_lnb:
            lnb_bc = const.tile([128, DIM], F32)
            nc.sync.dma_start(lnb_bc[:], lnbv.ap().unsqueeze(0).partition_broadcast(128))

        # ---- persistent activations ----
        pads = [big.tile([128, PADW, PADW], BF16, name=f"pad{g}") for g in range(3)]
        V0 = big.tile([128, L], BF16)
        V1 = big.tile([128, L], BF16)
        Vg = [V0, V1]
        Vt = big.tile([128, LCH, DIM], BF16)      # V^T, L-major, per-chunk
        BC = big.tile([128, L], BF16)             # B at parts 0:64, C at 64:128
        Bt = big.tile([128, LCH, DSTATE], BF16)   # B^T, L-major
        zt = big.tile([128, LCH, DIM], BF16)      # z, L-major
        dtraw = big.tile([16, L], BF16)
        dtt = big.tile([128, LCH, 16], BF16)
        dAsc = big.tile([128, LCH, NHEADS], BF16)
        kv_sb = big.tile([128, DIM], BF16)        # KV parked at partitions 64:128

        nc.gpsimd.memset(dtraw[:], 0.0)

        for _rep in range(reps):
            # border-only zeroing of the conv pads (interior fully overwritten)
            for g in range(3):
                p = pads[g][:]
                nc.gpsimd.memset(p[:, 0:1, :], 0.0)
                nc.gpsimd.memset(p[:, PADW - 1:PADW, :], 0.0)
                nc.gpsimd.memset(p[:, 1:PADW - 1, 0:1], 0.0)
                nc.gpsimd.memset(p[:, 1:PADW - 1, PADW - 1:PADW], 0.0)

            with tc.tile_pool(name="pconv", bufs=2, space="PSUM") as pconv:
                # ---- Phase 1: dt in_proj + dA chain; BC + z in_proj; conv(BC) ----
                with (
                    tc.tile_pool(name="p1a", bufs=2, space="PSUM") as p1a,
                    tc.tile_pool(name="pdt", bufs=2, space="PSUM") as pdt,
                    tc.tile_pool(name="daf", bufs=1) as daf_pool,
                    tc.tile_pool(name="cdgp", bufs=1) as cdgp,
                    tc.tile_pool(name="zcmp", bufs=1) as zcmp,
                ):
                    cdg2 = cdgp.tile([128, 9, 128], BF16, tag="cdg")
                    nc.sync.dma_start(cdg2[:], convdiag.ap()[:, 18:27, :])
                    zcm = zcmp.tile([128, 2, L], BF16, tag="zcm")

                    for n in range(NCH):
                        ps = pdt.tile([NHEADS, 512], F32)
                        for k in range(2):
                            nc.tensor.matmul(ps[:], wxb[:, k, CONV_DIM:],
                                             X[:, k, n * 512:(n + 1) * 512],
                                             start=(k == 0), stop=(k == 1))
                        nc.vector.tensor_copy(dtraw[0:NHEADS, n * 512:(n + 1) * 512], ps[:])
                    nc.sync.dma_start_transpose(dtt[:], dtraw[:])
                    # dA = Ln(Exp(dt + bias) + 1) * exp(A_log), one contiguous
                    # ACT block so only one Exp/Ln table set load happens.
                    dAf = daf_pool.tile([128, LCH, NHEADS], F32, tag="dAf")
                    nc.vector.tensor_add(
                        dAf[:], dtt[:, :, 0:NHEADS],
                        dtb_t[:].unsqueeze(1).to_broadcast([128, LCH, NHEADS]))
                    nc.scalar.activation(dAf[:], dAf[:], AF.Exp)
                    nc.scalar.activation(dAf[:], dAf[:], AF.Ln, bias=1.0)
                    nc.vector.tensor_mul(
                        dAsc[:], dAf[:],
                        ea_t[:].unsqueeze(1).to_broadcast([128, LCH, NHEADS]))

                    # BC in_proj -> pad buffer
                    for n in range(NCH):
                        ps = p1a.tile([128, 512], F32)
                        for k in range(2):
                            nc.tensor.matmul(ps[:], wxb[:, k, 2 * 128:3 * 128],
                                             X[:, k, n * 512:(n + 1) * 512],
                                             start=(k == 0), stop=(k == 1))
                        r0 = n * 8
                        nc.vector.tensor_copy(pads[2][:, r0 + 1:r0 + 9, 1:1 + W_],
                                              ps[:].rearrange("p (r w) -> p r w", w=W_))
                    # z in_proj, channel-major (stationary W_z slices)
                    for g in range(2):
                        for n in range(NCH):
                            ps = p1a.tile([128, 512], F32)
                            for k in range(2):
                                nc.tensor.matmul(ps[:], wz[:, k, g * 128:(g + 1) * 128],
                                                 X[:, k, n * 512:(n + 1) * 512],
                                                 start=(k == 0), stop=(k == 1))
                            if n % 2 == 0:
                                nc.vector.tensor_copy(zcm[:, g, n * 512:(n + 1) * 512], ps[:])
                            else:
                                nc.scalar.copy(zcm[:, g, n * 512:(n + 1) * 512], ps[:])
                    # conv(BC) + SiLU evacuation
                    for n in range(NCH):
                        pc = pconv.tile([128, 512], F32, tag="pc")
                        r0 = n * 8
                        for ti in range(9):
                            dy, dx = ti // 3, ti % 3
                            nc.tensor.matmul(
                                pc[:].rearrange("p (r w) -> p r w", w=W_),
                                cdg2[:, ti, :],
                                pads[2][:, r0 + dy:r0 + dy + 8, dx:dx + W_],
                                start=(ti == 0), stop=(ti == 8))
                        nc.scalar.activation(BC[:, n * 512:(n + 1) * 512], pc[:],
                                             AF.Silu, bias=cb[:, 2:3])
                    # B^T and z^T via XBAR (overlap phase 2 compute)
                    nc.sync.dma_start_transpose(Bt[:], BC[0:DSTATE, :])
                    for g in range(2):
                        nc.sync.dma_start_transpose(
                            zt[:, :, g * 128:(g + 1) * 128], zcm[:, g, :])

                # ---- Phase 2: xv in_proj + conv + V^T via XBAR + KV ----
                with (
                    tc.tile_pool(name="p1", bufs=2, space="PSUM") as p1,
                    tc.tile_pool(name="pkv", bufs=1, space="PSUM") as pkv,
                    tc.tile_pool(name="sbt", bufs=3) as sbt,
                    tc.tile_pool(name="cdgp2", bufs=2) as cdgp2,
                ):
                    kvp = pkv.tile([DSTATE, DIM], F32)
                    cdgv = []
                    for g in range(2):
                        c = cdgp2.tile([128, 9, 128], BF16, tag="cdg2")
                        nc.sync.dma_start(c[:], convdiag.ap()[:, g * 9:(g + 1) * 9, :])
                        cdgv.append(c)

                    def inproj_xv(n):
                        for g in range(2):
                            ps = p1.tile([128, 512], F32, tag="p1")
                            for k in range(2):
                                nc.tensor.matmul(ps[:], wxb[:, k, g * 128:(g + 1) * 128],
                                                 X[:, k, n * 512:(n + 1) * 512],
                                                 start=(k == 0), stop=(k == 1))
                            r0 = n * 8
                            dst = pads[g][:, r0 + 1:r0 + 9, 1:1 + W_]
                            src2 = ps[:].rearrange("p (r w) -> p r w", w=W_)
                            if g == 1:
                                nc.vector.tensor_copy(dst, src2)
                            else:
                                nc.scalar.copy(dst, src2)

                    def conv_v(n):
                        for g in range(2):
                            pc = pconv.tile([128, 512], F32, tag="pc")
                            r0 = n * 8
                            for ti in range(9):
                                dy, dx = ti // 3, ti % 3
                                nc.tensor.matmul(
                                    pc[:].rearrange("p (r w) -> p r w", w=W_),
                                    cdgv[g][:, ti, :],
                                    pads[g][:, r0 + dy:r0 + dy + 8, dx:dx + W_],
                                    start=(ti == 0), stop=(ti == 8))
                            nc.scalar.activation(Vg[g][:, n * 512:(n + 1) * 512], pc[:],
                                                 AF.Silu, bias=cb[:, g:g + 1])
                        nc.sync.dma_start_transpose(
                            Vt[:, 4 * n:4 * n + 4, 0:128], V0[:, n * 512:(n + 1) * 512])
                        nc.sync.dma_start_transpose(
                            Vt[:, 4 * n:4 * n + 4, 128:256], V1[:, n * 512:(n + 1) * 512])

                    def ssd(n):
                        for i in range(4 * n, 4 * n + 4):
                            vst = sbt.tile([128, DIM], BF16, tag="vst")
                            nc.vector.tensor_mul(
                                vst[:].rearrange("p (h q) -> p h q", h=NHEADS),
                                Vt[:, i, :].rearrange("p (h q) -> p h q", h=NHEADS),
                                dAsc[:, i, :].unsqueeze(2).to_broadcast([128, NHEADS, HEAD_DIM]),
                            )
                            nc.tensor.matmul(kvp[:], Bt[:, i, :], vst[:],
                                             start=(i == 0), stop=(i == LCH - 1))

                    for n in range(NCH + 2):
                        if n < NCH:
                            inproj_xv(n)
                        if 1 <= n < NCH + 1:
                            conv_v(n - 1)
                        if n >= 2:
                            ssd(n - 2)
                    nc.scalar.copy(kv_sb[DSTATE:2 * DSTATE, :], kvp[:])

            # ---- Phase 3: y, D-skip, LN, gate, XBAR transpose, out_proj ----
            with (
                tc.tile_pool(name="pyp", bufs=3, space="PSUM") as pyp,
                tc.tile_pool(name="pout", bufs=2, space="PSUM") as pout,
                tc.tile_pool(name="s6", bufs=5) as s6,
                tc.tile_pool(name="s6g", bufs=2) as s6g,
            ):
                yprs = [None] * (LCH // 2)
                yzt_grp = None

                def emit_front(p):
                    ypr = pyp.tile([128, 2, DIM], F32, tag="ypr")
                    for j in range(2):
                        i = 2 * p + j
                        sl = slice(i * 128, (i + 1) * 128)
                        nc.tensor.matmul(ypr[:, j, :], BC[DSTATE:2 * DSTATE, sl],
                                         kv_sb[DSTATE:2 * DSTATE, :], start=True, stop=True)
                    # D-skip: y += V * D, added straight into PSUM on DVE
                    if generic_d:
                        vtmp = s6.tile([128, 2, DIM], F32, tag="vtmp")
                        nc.vector.tensor_mul(
                            vtmp[:], Vt[:, 2 * p:2 * p + 2, :],
                            dex_bc[:].unsqueeze(1).to_broadcast([128, 2, DIM]))
                        nc.vector.tensor_add(ypr[:], ypr[:], vtmp[:])
                    else:
                        nc.vector.tensor_add(ypr[:], ypr[:], Vt[:, 2 * p:2 * p + 2, :])

                    st = s6.tile([128, 2, 6], F32, tag="st")
                    for j in range(2):
                        nc.vector.bn_stats(st[:, j, :], ypr[:, j, :])
                    mv = s6.tile([128, 2, 2], F32, tag="mv")
                    for j in range(2):
                        nc.vector.bn_aggr(mv[:, j, :], st[:, j, :])
                    rstd = s6.tile([128, 2], F32, tag="rstd")
                    nc.scalar.activation(rstd[:], mv[:, :, 1], AF.Abs_reciprocal_sqrt,
                                         bias=eps_t[:])
                    negm = s6.tile([128, 2], F32, tag="negm")
                    nc.vector.tensor_scalar_mul(negm[:], mv[:, :, 0], -1.0)
                    nmr = s6.tile([128, 2], F32, tag="nmr")
                    nc.vector.tensor_mul(nmr[:], negm[:], rstd[:])
                    yn = s6.tile([128, 2, DIM], F32, tag="yn")
                    for j in range(2):
                        nc.scalar.activation(yn[:, j, :], ypr[:, j, :], AF.Identity,
                                             bias=nmr[:, j:j + 1], scale=rstd[:, j:j + 1])
                    if has_lnw:
                        for j in range(2):
                            nc.vector.tensor_mul(yn[:, j, :], yn[:, j, :], lnw_bc[:])
                    if has_lnb:
                        for j in range(2):
                            nc.vector.tensor_add(yn[:, j, :], yn[:, j, :], lnb_bc[:])
                    yz = s6.tile([128, 2, DIM], BF16, tag="yz")
                    nc.vector.tensor_mul(yz[:], yn[:], zt[:, 2 * p:2 * p + 2, :])
                    yprs[p] = yz

                def emit_back(p):
                    nonlocal yzt_grp
                    yz = yprs[p]
                    if p % 2 == 0:
                        yzt_grp = s6g.tile([128, 2, 4, 128], BF16, tag="yzt")
                    for j in range(2):
                        jj = (p % 2) * 2 + j
                        nc.sync.dma_start_transpose(yzt_grp[:, :, jj, :], yz[:, j, :])
                    if p % 2 == 1:
                        n = p // 2
                        for mo in range(2):
                            po = pout.tile([128, 512], F32)
                            for k in range(2):
                                nc.tensor.matmul(po[:], wo[:, k, mo * 128:(mo + 1) * 128],
                                                 yzt_grp[:, k, :, :], start=(k == 0), stop=(k == 1))
                            ob = s6g.tile([128, 512], F32, tag="ob")
                            nc.scalar.copy(ob[:], po[:])
                            eng = nc.sync if (2 * n + mo) % 2 == 0 else nc.scalar
                            eng.dma_start(
                                outd.ap()[mo * 128:(mo + 1) * 128, n * 512:(n + 1) * 512],
                                ob[:])

                NP = LCH // 2
                PIPE = 3
                for p in range(NP + PIPE):
                    if p < NP:
                        emit_front(p)
                    if p >= PIPE:
                        emit_back(p - PIPE)


_CACHE = {}


def _prep(W_in, conv_w, conv_b, A_log, dt_bias, D, ln_w, ln_b, W_out):
    W_in = np.asarray(W_in, np.float32)
    conv_w = np.asarray(conv_w, np.float32)
    conv_b = np.asarray(conv_b, np.float32)
    A_log = np.asarray(A_log, np.float32)
    dt_bias = np.asarray(dt_bias, np.float32)
    D = np.asarray(D, np.float32)
    ln_w = np.asarray(ln_w, np.float32)
    ln_b = np.asarray(ln_b, np.float32)
    W_out = np.asarray(W_out, np.float32)

    WinT = np.ascontiguousarray(W_in.T)                       # [256, 648]
    w_xbcdt = np.ascontiguousarray(WinT[:, DIM:]).astype(BF)  # [256, 392]
    w_z = np.ascontiguousarray(WinT[:, :DIM]).astype(BF)      # [256, 256]
    w_out_t = np.ascontiguousarray(W_out.T).astype(BF)        # [256, 256]

    cd = np.zeros((128, 27, 128), np.float32)
    idx = np.arange(128)
    for g in range(3):
        for t in range(9):
            cd[idx, g * 9 + t, idx] = conv_w[g * 128 + idx, 0, t // 3, t % 3]

    cbm = np.zeros((128, 3), np.float32)
    for g in range(3):
        cbm[:, g] = conv_b[g * 128:(g + 1) * 128]

    flags = {
        "generic_d": not np.allclose(D, 1.0),
        "has_lnw": not np.allclose(ln_w, 1.0),
        "has_lnb": not np.allclose(ln_b, 0.0),
    }
    shared = dict(
        w_xbcdt=w_xbcdt, w_z=w_z, w_out=w_out_t, convdiag=cd.astype(BF),
        dtbias=dt_bias.reshape(NHEADS, 1).astype(np.float32),
        expa=np.exp(A_log).reshape(NHEADS, 1).astype(np.float32),
        convb=cbm, dexpand=np.repeat(D, HEAD_DIM).reshape(1, DIM).astype(np.float32),
        lnwv=ln_w, lnbv=ln_b,
    )
    return flags, shared


def _get_nc(flags, reps=1):
    key = (tuple(sorted(flags.items())), reps)
    if key not in _CACHE:
        _CACHE[key] = _build(flags, reps)
    return _CACHE[key]


def kernel(x, W_in, conv_w, conv_b, A_log, dt_bias, D, ln_w, ln_b, W_out,
           _trace=False):
    x = np.asarray(x, np.float32)
    flags, shared = _prep(W_in, conv_w, conv_b, A_log, dt_bias, D, ln_w, ln_b, W_out)
    nc = _get_nc(flags)
    xb = x.reshape(B_, DIM, L).astype(BF)
    in_maps = [dict(xb=np.ascontiguousarray(xb[b]), **shared) for b in range(B_)]
    res = run_bass_kernel_spmd(nc, in_maps, core_ids=list(range(NCORES)), trace=_trace)
    out = np.stack([res.results[b]["outb"] for b in range(B_)])
    out = out.reshape(B_, DIM, H_, W_)
    if _trace:
        return out, res
    return out


# revision 10
# speedup vs baseline: 1.0684x; 1.0340x over previous
# Bass/Tile TRN2 kernel for nn_BlqSSM (Mamba2-SSD-style block with depthwise
# 3x3 conv, non-causal linear attention, LayerNorm gate, out-projection).
#
# Sharding: data-parallel over batch — 8 batches on 8 NeuronCores, weights
# replicated, no collectives. Each core computes one full batch element.
#
# v2: bf16 datapath + XBAR DMA transposes. All PE-transpose matmuls from v1
# (V^T, B^T, dA^T, yz^T, D-skip) are replaced by dma_start_transpose (bf16
# only) or DVE adds, and activations/weights stream through the PE in bf16
# (1 cycle/row, half-size LDWEIGHTS). PSUM accumulation stays fp32.
#
# Per-core dataflow (L = H*W = 4096, channel-major = [C, L], L-major = [L, C]):
#   1. in_proj (PE, bf16): dt rows -> [16, L] (top 8 zero) -> XBAR transpose;
#      dA = Ln(Exp(dt + bias) + 1) * exp(A_log) in one contiguous ACT block
#      (exactly 3 act-table loads in the whole kernel). BC rows are written
#      into a zero-padded [128, 66, 66] conv buffer; z is computed
#      channel-major with stationary W_z slices and XBAR-transposed to
#      L-major during phase 2.
#   2. depthwise conv (PE, bf16): 9 taps as diag-weight matmuls accumulating
#      in PSUM over shifted views of the padded buffer; SiLU+bias fused into
#      the PSUM->SBUF evacuation on ACT, emitting bf16.
#   3. SSD (PE): per 512-col chunk, XBAR-transpose B and V to L-major;
#      V^T is scaled by dA on DVE (bf16); KV [64, 256] accumulates over all
#      32 chunks in one PSUM tile.
#   4. y = C @ KV per 128-row chunk; the D-skip term (y += V*D) is a DVE add
#      of the saved V^T directly into PSUM. LayerNorm (DVE bn_stats/bn_aggr
#      + ACT rsqrt) + gating with z; yz (bf16) is XBAR-transposed back to
#      channel-major and fed to out_proj (PE, bf16), output DMA'd as fp32.
import os
import sys

for _p in ("/opt/trn_rl_repo", os.path.expanduser("~/.axon_site/_ro/trn_rl_repo")):
    if os.path.isdir(_p) and _p not in sys.path:
        sys.path.insert(0, _p)

import numpy as np
import ml_dtypes

import concourse.bass as bass
import concourse.mybir as mybir
import concourse.tile as tile
from concourse import bacc
from concourse.bass_utils import run_bass_kernel_spmd

F32 = mybir.dt.float32
BF16 = mybir.dt.bfloat16
AF = mybir.ActivationFunctionType
ALU = mybir.AluOpType

DIM = 256
DSTATE = 64
NHEADS = 8
HEAD_DIM = DIM // NHEADS          # 32
CONV_DIM = DIM + 2 * DSTATE       # 384
D_IN_PROJ = 2 * DIM + 2 * DSTATE + NHEADS  # 648
LN_EPS = 1e-5
B_, H_, W_ = 8, 64, 64
L = H_ * W_                       # 4096
NCORES = 8
NCH = 8                           # 512-wide column chunks of L
LCH = 32                          # 128-wide column chunks of L
PADW = H_ + 2                     # 66

BF = ml_dtypes.bfloat16


def _build(flags, reps=1):
    generic_d = flags["generic_d"]
    has_lnw = flags["has_lnw"]
    has_lnb = flags["has_lnb"]

    nc = bacc.Bacc("TRN2", target_bir_lowering=False, debug=False, num_devices=NCORES)

    xd = nc.dram_tensor("xb", [DIM, L], BF16, kind="ExternalInput")
    w_xbcdt = nc.dram_tensor("w_xbcdt", [DIM, CONV_DIM + NHEADS], BF16, kind="ExternalInput")
    w_z = nc.dram_tensor("w_z", [DIM, DIM], BF16, kind="ExternalInput")
    w_out = nc.dram_tensor("w_out", [DIM, DIM], BF16, kind="ExternalInput")
    convdiag = nc.dram_tensor("convdiag", [128, 27, 128], BF16, kind="ExternalInput")
    dtbias = nc.dram_tensor("dtbias", [NHEADS, 1], F32, kind="ExternalInput")
    expa = nc.dram_tensor("expa", [NHEADS, 1], F32, kind="ExternalInput")
    convb = nc.dram_tensor("convb", [128, 3], F32, kind="ExternalInput")
    dexpand = nc.dram_tensor("dexpand", [1, DIM], F32, kind="ExternalInput")
    lnwv = nc.dram_tensor("lnwv", [DIM], F32, kind="ExternalInput")
    lnbv = nc.dram_tensor("lnbv", [DIM], F32, kind="ExternalInput")
    outd = nc.dram_tensor("outb", [DIM, L], F32, kind="ExternalOutput")

    with tile.TileContext(nc) as tc:
        _emit(nc, tc, locals(), generic_d, has_lnw, has_lnb, reps)
    nc.compile()
    return nc


def _emit(nc, tc, t, generic_d, has_lnw, has_lnb, reps=1):
    from contextlib import ExitStack

    xd, w_xbcdt, w_z, w_out = t["xd"], t["w_xbcdt"], t["w_z"], t["w_out"]
    convdiag, dtbias, expa = t["convdiag"], t["dtbias"], t["expa"]
    convb, dexpand, lnwv, lnbv, outd = t["convb"], t["dexpand"], t["lnwv"], t["lnbv"], t["outd"]

    with ExitStack() as ctx:
        const = ctx.enter_context(tc.tile_pool(name="const", bufs=1))
        big = ctx.enter_context(tc.tile_pool(name="big", bufs=1))

        # ---- weights + X, ordered so the first matmuls unblock earliest ----
        wxb = const.tile([128, 2, CONV_DIM + NHEADS], BF16)
        nc.sync.dma_start(wxb[:], w_xbcdt.ap().rearrange("(t k) m -> k t m", k=128))
        X = big.tile([128, 2, L], BF16)
        xr = xd.ap().rearrange("(t k) l -> k t l", k=128)
        for c in range(4):
            for k in range(2):
                eng = nc.sync if k == 0 else nc.scalar
                eng.dma_start(X[:, k, c * 1024:(c + 1) * 1024], xr[:, k, c * 1024:(c + 1) * 1024])
        dtb_t = const.tile([128, NHEADS], F32)
        nc.sync.dma_start(dtb_t[:], dtbias.ap().rearrange("a b -> b a").partition_broadcast(128))
        ea_t = const.tile([128, NHEADS], F32)
        nc.sync.dma_start(ea_t[:], expa.ap().rearrange("a b -> b a").partition_broadcast(128))
        cb = const.tile([128, 3], F32)
        nc.sync.dma_start(cb[:], convb.ap())
        eps_t = const.tile([128, 1], F32)
        nc.vector.memset(eps_t[:], LN_EPS)
        wz = const.tile([128, 2, DIM], BF16)
        nc.scalar.dma_start(wz[:], w_z.ap().rearrange("(t k) m -> k t m", k=128))
        wo = const.tile([128, 2, DIM], BF16)
        nc.scalar.dma_start(wo[:], w_out.ap().rearrange("(t k) m -> k t m", k=128))
        if generic_d:
            dex_bc = const.tile([128, DIM], F32)
            nc.sync.dma_start(dex_bc[:], dexpand.ap().partition_broadcast(128))
        if has_lnw:
            lnw_bc = const.tile([128, DIM], F32)
            nc.sync.dma_start(lnw_bc[:], lnwv.ap().unsqueeze(0).partition_broadcast(128))
        if has_lnb:
            lnb_bc = const.tile([128, DIM], F32)
            nc.sync.dma_start(lnb_bc[:], lnbv.ap().unsqueeze(0).partition_broadcast(128))

        # ---- persistent activations ----
        pads = [big.tile([128, PADW, PADW], BF16, name=f"pad{g}") for g in range(3)]
        V0 = big.tile([128, L], BF16)
        V1 = big.tile([128, L], BF16)
        Vg = [V0, V1]
        Vt = big.tile([128, LCH, DIM], BF16)      # V^T, L-major, per-chunk
        BC = big.tile([128, L], BF16)             # B at parts 0:64, C at 64:128
        Bt = big.tile([128, LCH, DSTATE], BF16)   # B^T, L-major
        zt = big.tile([128, LCH, DIM], BF16)      # z, L-major
        dtraw = big.tile([16, L], BF16)
        dtt = big.tile([128, LCH, 16], BF16)
        dAsc = big.tile([128, LCH, NHEADS], BF16)
        kv_sb = big.tile([128, DIM], BF16)        # KV parked at partitions 64:128

        nc.gpsimd.memset(dtraw[:], 0.0)

        for _rep in range(reps):
            # border-only zeroing of the conv pads (interior fully overwritten)
            for g in range(3):
                p = pads[g][:]
                nc.gpsimd.memset(p[:, 0:1, :], 0.0)
                nc.gpsimd.memset(p[:, PADW - 1:PADW, :], 0.0)
                nc.gpsimd.memset(p[:, 1:PADW - 1, 0:1], 0.0)
                nc.gpsimd.memset(p[:, 1:PADW - 1, PADW - 1:PADW], 0.0)

            with tc.tile_pool(name="pconv", bufs=2, space="PSUM") as pconv:
                # ---- Phase 1: dt in_proj + dA chain; BC + z in_proj; conv(BC) ----
                with (
                    tc.tile_pool(name="p1a", bufs=2, space="PSUM") as p1a,
                    tc.tile_pool(name="pdt", bufs=2, space="PSUM") as pdt,
                    tc.tile_pool(name="daf", bufs=1) as daf_pool,
                    tc.tile_pool(name="cdgp", bufs=1) as cdgp,
                    tc.tile_pool(name="zcmp", bufs=1) as zcmp,
                ):
                    cdg2 = cdgp.tile([128, 9, 128], BF16, tag="cdg")
                    nc.sync.dma_start(cdg2[:], convdiag.ap()[:, 18:27, :])
                    zcm = zcmp.tile([128, 2, L], BF16, tag="zcm")

                    for n in range(NCH):
                        ps = pdt.tile([NHEADS, 512], F32)
                        for k in range(2):
                            nc.tensor.matmul(ps[:], wxb[:, k, CONV_DIM:],
                                             X[:, k, n * 512:(n + 1) * 512],
                                             start=(k == 0), stop=(k == 1))
                        nc.vector.tensor_copy(dtraw[0:NHEADS, n * 512:(n + 1) * 512], ps[:])
                    nc.sync.dma_start_transpose(dtt[:], dtraw[:])
                    # dA = Ln(Exp(dt + bias) + 1) * exp(A_log), one contiguous
                    # ACT block so only one Exp/Ln table set load happens.
                    dAf = daf_pool.tile([128, LCH, NHEADS], F32, tag="dAf")
                    nc.vector.tensor_add(
                        dAf[:], dtt[:, :, 0:NHEADS],
                        dtb_t[:].unsqueeze(1).to_broadcast([128, LCH, NHEADS]))
                    nc.scalar.activation(dAf[:], dAf[:], AF.Exp)
                    nc.scalar.activation(dAf[:], dAf[:], AF.Ln, bias=1.0)
                    nc.vector.tensor_mul(
                        dAsc[:], dAf[:],
                        ea_t[:].unsqueeze(1).to_broadcast([128, LCH, NHEADS]))

                    # BC in_proj -> pad buffer
                    for n in range(NCH):
                        ps = p1a.tile([128, 512], F32)
                        for k in range(2):
                            nc.tensor.matmul(ps[:], wxb[:, k, 2 * 128:3 * 128],
                                             X[:, k, n * 512:(n + 1) * 512],
                                             start=(k == 0), stop=(k == 1))
                        r0 = n * 8
                        nc.vector.tensor_copy(pads[2][:, r0 + 1:r0 + 9, 1:1 + W_],
                                              ps[:].rearrange("p (r w) -> p r w", w=W_))
                    # z in_proj, channel-major (stationary W_z slices)
                    for g in range(2):
                        for n in range(NCH):
                            ps = p1a.tile([128, 512], F32)
                            for k in range(2):
                                nc.tensor.matmul(ps[:], wz[:, k, g * 128:(g + 1) * 128],
                                                 X[:, k, n * 512:(n + 1) * 512],
                                                 start=(k == 0), stop=(k == 1))
                            if n % 2 == 0:
                                nc.vector.tensor_copy(zcm[:, g, n * 512:(n + 1) * 512], ps[:])
                            else:
                                nc.scalar.copy(zcm[:, g, n * 512:(n + 1) * 512], ps[:])
                    # conv(BC) + SiLU evacuation
                    for n in range(NCH):
                        pc = pconv.tile([128, 512], F32, tag="pc")
                        r0 = n * 8
                        for ti in range(9):
                            dy, dx = ti // 3, ti % 3
                            nc.tensor.matmul(
                                pc[:].rearrange("p (r w) -> p r w", w=W_),
                                cdg2[:, ti, :],
                                pads[2][:, r0 + dy:r0 + dy + 8, dx:dx + W_],
                                start=(ti == 0), stop=(ti == 8))
                        nc.scalar.activation(BC[:, n * 512:(n + 1) * 512], pc[:],
                                             AF.Silu, bias=cb[:, 2:3])
                    # B^T and z^T via XBAR (overlap phase 2 compute)
                    nc.sync.dma_start_transpose(Bt[:], BC[0:DSTATE, :])
                    for g in range(2):
                        nc.scalar.dma_start_transpose(
                            zt[:, :, g * 128:(g + 1) * 128], zcm[:, g, :])

                # ---- Phase 2: xv in_proj + conv + V^T via XBAR + KV ----
                with (
                    tc.tile_pool(name="p1", bufs=2, space="PSUM") as p1,
                    tc.tile_pool(name="pkv", bufs=1, space="PSUM") as pkv,
                    tc.tile_pool(name="sbt", bufs=3) as sbt,
                    tc.tile_pool(name="cdgp2", bufs=2) as cdgp2,
                ):
                    kvp = pkv.tile([DSTATE, DIM], F32)
                    cdgv = []
                    for g in range(2):
                        c = cdgp2.tile([128, 9, 128], BF16, tag="cdg2")
                        nc.sync.dma_start(c[:], convdiag.ap()[:, g * 9:(g + 1) * 9, :])
                        cdgv.append(c)

                    def inproj_xv(n):
                        for g in range(2):
                            ps = p1.tile([128, 512], F32, tag="p1")
                            for k in range(2):
                                nc.tensor.matmul(ps[:], wxb[:, k, g * 128:(g + 1) * 128],
                                                 X[:, k, n * 512:(n + 1) * 512],
                                                 start=(k == 0), stop=(k == 1))
                            r0 = n * 8
                            dst = pads[g][:, r0 + 1:r0 + 9, 1:1 + W_]
                            src2 = ps[:].rearrange("p (r w) -> p r w", w=W_)
                            if g == 1:
                                nc.vector.tensor_copy(dst, src2)
                            else:
                                nc.scalar.copy(dst, src2)

                    def conv_v(n):
                        for g in range(2):
                            pc = pconv.tile([128, 512], F32, tag="pc")
                            r0 = n * 8
                            for ti in range(9):
                                dy, dx = ti // 3, ti % 3
                                nc.tensor.matmul(
                                    pc[:].rearrange("p (r w) -> p r w", w=W_),
                                    cdgv[g][:, ti, :],
                                    pads[g][:, r0 + dy:r0 + dy + 8, dx:dx + W_],
                                    start=(ti == 0), stop=(ti == 8))
                            nc.scalar.activation(Vg[g][:, n * 512:(n + 1) * 512], pc[:],
                                                 AF.Silu, bias=cb[:, g:g + 1])
                        if n % 2 == 1:
                            m = n // 2
                            for g in range(2):
                                eng = nc.sync if g == 0 else nc.scalar
                                eng.dma_start_transpose(
                                    Vt[:, 8 * m:8 * m + 8, g * 128:(g + 1) * 128],
                                    Vg[g][:, m * 1024:(m + 1) * 1024])

                    def ssd(m):
                        for i in range(8 * m, 8 * m + 8):
                            vst = sbt.tile([128, DIM], BF16, tag="vst")
                            nc.vector.tensor_mul(
                                vst[:].rearrange("p (h q) -> p h q", h=NHEADS),
                                Vt[:, i, :].rearrange("p (h q) -> p h q", h=NHEADS),
                                dAsc[:, i, :].unsqueeze(2).to_broadcast([128, NHEADS, HEAD_DIM]),
                            )
                            nc.tensor.matmul(kvp[:], Bt[:, i, :], vst[:],
                                             start=(i == 0), stop=(i == LCH - 1))

                    for n in range(NCH + 2):
                        if n < NCH:
                            inproj_xv(n)
                        if 1 <= n < NCH + 1:
                            conv_v(n - 1)
                        if n >= 3 and (n - 3) % 2 == 0:
                            ssd((n - 3) // 2)
                    nc.scalar.copy(kv_sb[DSTATE:2 * DSTATE, :], kvp[:])

            # ---- Phase 3: y, D-skip, LN, gate, XBAR transpose, out_proj ----
            with (
                tc.tile_pool(name="pyp", bufs=3, space="PSUM") as pyp,
                tc.tile_pool(name="pout", bufs=2, space="PSUM") as pout,
                tc.tile_pool(name="s6", bufs=5) as s6,
                tc.tile_pool(name="s6g", bufs=2) as s6g,
            ):
                yprs = [None] * (LCH // 2)
                yzt_grp = None

                def emit_front(p):
                    ypr = pyp.tile([128, 2, DIM], F32, tag="ypr")
                    for j in range(2):
                        i = 2 * p + j
                        sl = slice(i * 128, (i + 1) * 128)
                        nc.tensor.matmul(ypr[:, j, :], BC[DSTATE:2 * DSTATE, sl],
                                         kv_sb[DSTATE:2 * DSTATE, :], start=True, stop=True)
                    # D-skip: y += V * D, added straight into PSUM on DVE
                    if generic_d:
                        vtmp = s6.tile([128, 2, DIM], F32, tag="vtmp")
                        nc.vector.tensor_mul(
                            vtmp[:], Vt[:, 2 * p:2 * p + 2, :],
                            dex_bc[:].unsqueeze(1).to_broadcast([128, 2, DIM]))
                        nc.vector.tensor_add(ypr[:], ypr[:], vtmp[:])
                    else:
                        nc.vector.tensor_add(ypr[:], ypr[:], Vt[:, 2 * p:2 * p + 2, :])

                    st = s6.tile([128, 2, 6], F32, tag="st")
                    for j in range(2):
                        nc.vector.bn_stats(st[:, j, :], ypr[:, j, :])
                    mv = s6.tile([128, 2, 2], F32, tag="mv")
                    for j in range(2):
                        nc.vector.bn_aggr(mv[:, j, :], st[:, j, :])
                    rstd = s6.tile([128, 2], F32, tag="rstd")
                    nc.scalar.activation(rstd[:], mv[:, :, 1], AF.Abs_reciprocal_sqrt,
                                         bias=eps_t[:])
                    negm = s6.tile([128, 2], F32, tag="negm")
                    nc.vector.tensor_scalar_mul(negm[:], mv[:, :, 0], -1.0)
                    nmr = s6.tile([128, 2], F32, tag="nmr")
                    nc.vector.tensor_mul(nmr[:], negm[:], rstd[:])
                    yn = s6.tile([128, 2, DIM], F32, tag="yn")
                    for j in range(2):
                        nc.scalar.activation(yn[:, j, :], ypr[:, j, :], AF.Identity,
                                             bias=nmr[:, j:j + 1], scale=rstd[:, j:j + 1])
                    if has_lnw:
                        for j in range(2):
                            nc.vector.tensor_mul(yn[:, j, :], yn[:, j, :], lnw_bc[:])
                    if has_lnb:
                        for j in range(2):
                            nc.vector.tensor_add(yn[:, j, :], yn[:, j, :], lnb_bc[:])
                    yz = s6.tile([128, 2, DIM], BF16, tag="yz")
                    nc.vector.tensor_mul(yz[:], yn[:], zt[:, 2 * p:2 * p + 2, :])
                    yprs[p] = yz

                outr = outd.ap().rearrange("(m k) l -> k m l", k=128)

                def emit_back(p):
                    nonlocal yzt_grp
                    yz = yprs[p]
                    if p % 2 == 0:
                        # layout [cc, jj(chunk), g(k-half), l]; one XBAR call per
                        # pair writes slots (j, g) for both chunks of the pair
                        yzt_grp = s6g.tile([128, 4, 2, 128], BF16, tag="yzt")
                    jj0 = (p % 2) * 2
                    eng = nc.sync if p % 2 == 0 else nc.scalar
                    eng.dma_start_transpose(
                        yzt_grp[:, jj0:jj0 + 2, :, :],
                        yz[:].rearrange("a b c -> a (b c)"))
                    if p % 2 == 1:
                        n = p // 2
                        ob = s6g.tile([128, 2, 512], F32, tag="ob")
                        for mo in range(2):
                            po = pout.tile([128, 512], F32)
                            for k in range(2):
                                nc.tensor.matmul(po[:], wo[:, k, mo * 128:(mo + 1) * 128],
                                                 yzt_grp[:, :, k, :], start=(k == 0), stop=(k == 1))
                            if mo == 0:
                                nc.vector.tensor_copy(ob[:, mo, :], po[:])
                            else:
                                nc.scalar.copy(ob[:, mo, :], po[:])
                        eng = nc.sync if n % 2 == 0 else nc.scalar
                        eng.dma_start(outr[:, :, n * 512:(n + 1) * 512], ob[:])

                NP = LCH // 2
                PIPE = 3
                for p in range(NP + PIPE):
                    if p < NP:
                        emit_front(p)
                    if p >= PIPE:
                        emit_back(p - PIPE)


_CACHE = {}


def _prep(W_in, conv_w, conv_b, A_log, dt_bias, D, ln_w, ln_b, W_out):
    W_in = np.asarray(W_in, np.float32)
    conv_w = np.asarray(conv_w, np.float32)
    conv_b = np.asarray(conv_b, np.float32)
    A_log = np.asarray(A_log, np.float32)
    dt_bias = np.asarray(dt_bias, np.float32)
    D = np.asarray(D, np.float32)
    ln_w = np.asarray(ln_w, np.float32)
    ln_b = np.asarray(ln_b, np.float32)
    W_out = np.asarray(W_out, np.float32)

    WinT = np.ascontiguousarray(W_in.T)                       # [256, 648]
    w_xbcdt = np.ascontiguousarray(WinT[:, DIM:]).astype(BF)  # [256, 392]
    w_z = np.ascontiguousarray(WinT[:, :DIM]).astype(BF)      # [256, 256]
    w_out_t = np.ascontiguousarray(W_out.T).astype(BF)        # [256, 256]

    cd = np.zeros((128, 27, 128), np.float32)
    idx = np.arange(128)
    for g in range(3):
        for t in range(9):
            cd[idx, g * 9 + t, idx] = conv_w[g * 128 + idx, 0, t // 3, t % 3]

    cbm = np.zeros((128, 3), np.float32)
    for g in range(3):
        cbm[:, g] = conv_b[g * 128:(g + 1) * 128]

    flags = {
        "generic_d": not np.allclose(D, 1.0),
        "has_lnw": not np.allclose(ln_w, 1.0),
        "has_lnb": not np.allclose(ln_b, 0.0),
    }
    shared = dict(
        w_xbcdt=w_xbcdt, w_z=w_z, w_out=w_out_t, convdiag=cd.astype(BF),
        dtbias=dt_bias.reshape(NHEADS, 1).astype(np.float32),
        expa=np.exp(A_log).reshape(NHEADS, 1).astype(np.float32),
        convb=cbm, dexpand=np.repeat(D, HEAD_DIM).reshape(1, DIM).astype(np.float32),
        lnwv=ln_w, lnbv=ln_b,
    )
    return flags, shared


def _get_nc(flags, reps=1):
    key = (tuple(sorted(flags.items())), reps)
    if key not in _CACHE:
        _CACHE[key] = _build(flags, reps)
    return _CACHE[key]


def kernel(x, W_in, conv_w, conv_b, A_log, dt_bias, D, ln_w, ln_b, W_out,
           _trace=False):
    x = np.asarray(x, np.float32)
    flags, shared = _prep(W_in, conv_w, conv_b, A_log, dt_bias, D, ln_w, ln_b, W_out)
    nc = _get_nc(flags)
    xb = x.reshape(B_, DIM, L).astype(BF)
    in_maps = [dict(xb=np.ascontiguousarray(xb[b]), **shared) for b in range(B_)]
    res = run_bass_kernel_spmd(nc, in_maps, core_ids=list(range(NCORES)), trace=_trace)
    out = np.stack([res.results[b]["outb"] for b in range(B_)])
    out = out.reshape(B_, DIM, H_, W_)
    if _trace:
        return out, res
    return out


# revision 12
# speedup vs baseline: 1.1516x; 1.0779x over previous
# Bass/Tile TRN2 kernel for nn_BlqSSM (Mamba2-SSD-style block with depthwise
# 3x3 conv, non-causal linear attention, LayerNorm gate, out-projection).
#
# Sharding: data-parallel over batch — 8 batches on 8 NeuronCores, weights
# replicated, no collectives. Each core computes one full batch element.
#
# v2: bf16 datapath + XBAR DMA transposes. All PE-transpose matmuls from v1
# (V^T, B^T, dA^T, yz^T, D-skip) are replaced by dma_start_transpose (bf16
# only) or DVE adds, and activations/weights stream through the PE in bf16
# (1 cycle/row, half-size LDWEIGHTS). PSUM accumulation stays fp32.
#
# Per-core dataflow (L = H*W = 4096, channel-major = [C, L], L-major = [L, C]):
#   1. in_proj (PE, bf16): dt rows -> [16, L] (top 8 zero) -> XBAR transpose;
#      dA = Ln(Exp(dt + bias) + 1) * exp(A_log) in one contiguous ACT block
#      (exactly 3 act-table loads in the whole kernel). BC rows are written
#      into a zero-padded [128, 66, 66] conv buffer; z is computed
#      channel-major with stationary W_z slices and XBAR-transposed to
#      L-major during phase 2.
#   2. depthwise conv (PE, bf16): 9 taps as diag-weight matmuls accumulating
#      in PSUM over shifted views of the padded buffer; SiLU+bias fused into
#      the PSUM->SBUF evacuation on ACT, emitting bf16.
#   3. SSD (PE): per 512-col chunk, XBAR-transpose B and V to L-major;
#      V^T is scaled by dA on DVE (bf16); KV [64, 256] accumulates over all
#      32 chunks in one PSUM tile.
#   4. y = C @ KV per 128-row chunk; the D-skip term (y += V*D) is a DVE add
#      of the saved V^T directly into PSUM. LayerNorm (DVE bn_stats/bn_aggr
#      + ACT rsqrt) + gating with z; yz (bf16) is XBAR-transposed back to
#      channel-major and fed to out_proj (PE, bf16), output DMA'd as fp32.
import os
import sys

for _p in ("/opt/trn_rl_repo", os.path.expanduser("~/.axon_site/_ro/trn_rl_repo")):
    if os.path.isdir(_p) and _p not in sys.path:
        sys.path.insert(0, _p)

import numpy as np
import ml_dtypes

import concourse.bass as bass
import concourse.mybir as mybir
import concourse.tile as tile
from concourse import bacc
from concourse.bass_utils import run_bass_kernel_spmd

F32 = mybir.dt.float32
BF16 = mybir.dt.bfloat16
AF = mybir.ActivationFunctionType
ALU = mybir.AluOpType

DIM = 256
DSTATE = 64
NHEADS = 8
HEAD_DIM = DIM // NHEADS          # 32
CONV_DIM = DIM + 2 * DSTATE       # 384
D_IN_PROJ = 2 * DIM + 2 * DSTATE + NHEADS  # 648
LN_EPS = 1e-5
B_, H_, W_ = 8, 64, 64
L = H_ * W_                       # 4096
NCORES = 8
NCH = 8                           # 512-wide column chunks of L
LCH = 32                          # 128-wide column chunks of L
PADW = H_ + 2                     # 66

BF = ml_dtypes.bfloat16


def _build(flags, reps=1):
    generic_d = flags["generic_d"]
    has_lnw = flags["has_lnw"]
    has_lnb = flags["has_lnb"]

    nc = bacc.Bacc("TRN2", target_bir_lowering=False, debug=False, num_devices=NCORES)

    xd = nc.dram_tensor("xb", [DIM, L], BF16, kind="ExternalInput")
    w_xbcdt = nc.dram_tensor("w_xbcdt", [DIM, CONV_DIM + NHEADS], BF16, kind="ExternalInput")
    w_z = nc.dram_tensor("w_z", [DIM, DIM], BF16, kind="ExternalInput")
    w_out = nc.dram_tensor("w_out", [DIM, DIM], BF16, kind="ExternalInput")
    convdiag = nc.dram_tensor("convdiag", [128, 27, 128], BF16, kind="ExternalInput")
    dtbias = nc.dram_tensor("dtbias", [NHEADS, 1], F32, kind="ExternalInput")
    expa = nc.dram_tensor("expa", [NHEADS, 1], F32, kind="ExternalInput")
    convb = nc.dram_tensor("convb", [128, 3], F32, kind="ExternalInput")
    dexpand = nc.dram_tensor("dexpand", [1, DIM], F32, kind="ExternalInput")
    lnwv = nc.dram_tensor("lnwv", [DIM], F32, kind="ExternalInput")
    lnbv = nc.dram_tensor("lnbv", [DIM], F32, kind="ExternalInput")
    outd = nc.dram_tensor("outb", [DIM, L], F32, kind="ExternalOutput")

    with tile.TileContext(nc) as tc:
        _emit(nc, tc, locals(), generic_d, has_lnw, has_lnb, reps)
    nc.compile()
    return nc


def _emit(nc, tc, t, generic_d, has_lnw, has_lnb, reps=1):
    from contextlib import ExitStack

    xd, w_xbcdt, w_z, w_out = t["xd"], t["w_xbcdt"], t["w_z"], t["w_out"]
    convdiag, dtbias, expa = t["convdiag"], t["dtbias"], t["expa"]
    convb, dexpand, lnwv, lnbv, outd = t["convb"], t["dexpand"], t["lnwv"], t["lnbv"], t["outd"]

    with ExitStack() as ctx:
        const = ctx.enter_context(tc.tile_pool(name="const", bufs=1))
        big = ctx.enter_context(tc.tile_pool(name="big", bufs=1))

        # ---- weights + X, ordered so the first matmuls unblock earliest ----
        wxb = const.tile([128, 2, CONV_DIM + NHEADS], BF16)
        nc.sync.dma_start(wxb[:], w_xbcdt.ap().rearrange("(t k) m -> k t m", k=128))
        X = big.tile([128, 2, L], BF16)
        xr = xd.ap().rearrange("(t k) l -> k t l", k=128)
        for c in range(4):
            for k in range(2):
                eng = nc.sync if k == 0 else nc.scalar
                eng.dma_start(X[:, k, c * 1024:(c + 1) * 1024], xr[:, k, c * 1024:(c + 1) * 1024])
        dtb_t = const.tile([128, NHEADS], F32)
        nc.sync.dma_start(dtb_t[:], dtbias.ap().rearrange("a b -> b a").partition_broadcast(128))
        ea_t = const.tile([128, NHEADS], F32)
        nc.sync.dma_start(ea_t[:], expa.ap().rearrange("a b -> b a").partition_broadcast(128))
        cb = const.tile([128, 3], F32)
        nc.sync.dma_start(cb[:], convb.ap())
        eps_t = const.tile([128, 1], F32)
        nc.vector.memset(eps_t[:], LN_EPS)
        wz = const.tile([128, 2, DIM], BF16)
        nc.scalar.dma_start(wz[:], w_z.ap().rearrange("(t k) m -> k t m", k=128))
        wo = const.tile([128, 2, DIM], BF16)
        nc.scalar.dma_start(wo[:], w_out.ap().rearrange("(t k) m -> k t m", k=128))
        if generic_d:
            dex_bc = const.tile([128, DIM], F32)
            nc.sync.dma_start(dex_bc[:], dexpand.ap().partition_broadcast(128))
        if has_lnw:
            lnw_bc = const.tile([128, DIM], F32)
            nc.sync.dma_start(lnw_bc[:], lnwv.ap().unsqueeze(0).partition_broadcast(128))
        if has_lnb:
            lnb_bc = const.tile([128, DIM], F32)
            nc.sync.dma_start(lnb_bc[:], lnbv.ap().unsqueeze(0).partition_broadcast(128))

        # ---- persistent activations ----
        pads = [big.tile([128, PADW, PADW], BF16, name=f"pad{g}") for g in range(3)]
        V0 = big.tile([128, L], BF16)
        V1 = big.tile([128, L], BF16)
        Vg = [V0, V1]
        Vt = big.tile([128, LCH, DIM], BF16)      # V^T, L-major, per-chunk
        BC = big.tile([128, L], BF16)             # B at parts 0:64, C at 64:128
        Bt = big.tile([128, LCH, DSTATE], BF16)   # B^T, L-major
        zt = big.tile([128, LCH, DIM], BF16)      # z, L-major
        dtraw = big.tile([16, L], BF16)
        dtt = big.tile([128, LCH, 16], BF16)
        dAsc = big.tile([128, LCH, NHEADS], BF16)
        kv_sb = big.tile([128, DIM], BF16)        # KV parked at partitions 64:128

        nc.gpsimd.memset(dtraw[:], 0.0)

        for _rep in range(reps):
            # border-only zeroing of the conv pads (interior fully overwritten)
            for g in range(3):
                p = pads[g][:]
                nc.gpsimd.memset(p[:, 0:1, :], 0.0)
                nc.gpsimd.memset(p[:, PADW - 1:PADW, :], 0.0)
                nc.gpsimd.memset(p[:, 1:PADW - 1, 0:1], 0.0)
                nc.gpsimd.memset(p[:, 1:PADW - 1, PADW - 1:PADW], 0.0)

            with tc.tile_pool(name="pconv", bufs=2, space="PSUM") as pconv:
                # ---- Phase 1: dt in_proj + dA chain; BC + z in_proj; conv(BC) ----
                with (
                    tc.tile_pool(name="p1a", bufs=2, space="PSUM") as p1a,
                    tc.tile_pool(name="pdt", bufs=2, space="PSUM") as pdt,
                    tc.tile_pool(name="daf", bufs=1) as daf_pool,
                    tc.tile_pool(name="cdgp", bufs=1) as cdgp,
                    tc.tile_pool(name="zcmp", bufs=1) as zcmp,
                ):
                    cdg2 = cdgp.tile([128, 9, 128], BF16, tag="cdg")
                    nc.sync.dma_start(cdg2[:], convdiag.ap()[:, 18:27, :])
                    zcm = zcmp.tile([128, 2, L], BF16, tag="zcm")

                    for n in range(NCH):
                        ps = pdt.tile([NHEADS, 512], F32)
                        for k in range(2):
                            nc.tensor.matmul(ps[:], wxb[:, k, CONV_DIM:],
                                             X[:, k, n * 512:(n + 1) * 512],
                                             start=(k == 0), stop=(k == 1))
                        nc.vector.tensor_copy(dtraw[0:NHEADS, n * 512:(n + 1) * 512], ps[:])
                    nc.sync.dma_start_transpose(dtt[:], dtraw[:])
                    # dA = Ln(Exp(dt + bias) + 1) * exp(A_log), one contiguous
                    # ACT block so only one Exp/Ln table set load happens.
                    dAf = daf_pool.tile([128, LCH, NHEADS], F32, tag="dAf")
                    nc.vector.tensor_add(
                        dAf[:], dtt[:, :, 0:NHEADS],
                        dtb_t[:].unsqueeze(1).to_broadcast([128, LCH, NHEADS]))
                    nc.scalar.activation(dAf[:], dAf[:], AF.Exp)
                    nc.scalar.activation(dAf[:], dAf[:], AF.Ln, bias=1.0)
                    nc.vector.tensor_mul(
                        dAsc[:], dAf[:],
                        ea_t[:].unsqueeze(1).to_broadcast([128, LCH, NHEADS]))

                    # BC in_proj -> pad buffer
                    for n in range(NCH):
                        ps = p1a.tile([128, 512], F32)
                        for k in range(2):
                            nc.tensor.matmul(ps[:], wxb[:, k, 2 * 128:3 * 128],
                                             X[:, k, n * 512:(n + 1) * 512],
                                             start=(k == 0), stop=(k == 1))
                        r0 = n * 8
                        nc.vector.tensor_copy(pads[2][:, r0 + 1:r0 + 9, 1:1 + W_],
                                              ps[:].rearrange("p (r w) -> p r w", w=W_))
                    # z in_proj, channel-major (stationary W_z slices)
                    for g in range(2):
                        for n in range(NCH):
                            ps = p1a.tile([128, 512], F32)
                            for k in range(2):
                                nc.tensor.matmul(ps[:], wz[:, k, g * 128:(g + 1) * 128],
                                                 X[:, k, n * 512:(n + 1) * 512],
                                                 start=(k == 0), stop=(k == 1))
                            if n % 2 == 0:
                                nc.vector.tensor_copy(zcm[:, g, n * 512:(n + 1) * 512], ps[:])
                            else:
                                nc.scalar.copy(zcm[:, g, n * 512:(n + 1) * 512], ps[:])
                    # conv(BC) + SiLU evacuation
                    for n in range(NCH):
                        pc = pconv.tile([128, 512], F32, tag="pc")
                        r0 = n * 8
                        for ti in range(9):
                            dy, dx = ti // 3, ti % 3
                            nc.tensor.matmul(
                                pc[:].rearrange("p (r w) -> p r w", w=W_),
                                cdg2[:, ti, :],
                                pads[2][:, r0 + dy:r0 + dy + 8, dx:dx + W_],
                                start=(ti == 0), stop=(ti == 8))
                        nc.scalar.activation(BC[:, n * 512:(n + 1) * 512], pc[:],
                                             AF.Silu, bias=cb[:, 2:3])
                    # B^T and z^T via XBAR (overlap phase 2 compute)
                    nc.sync.dma_start_transpose(Bt[:], BC[0:DSTATE, :])
                    for g in range(2):
                        nc.scalar.dma_start_transpose(
                            zt[:, :, g * 128:(g + 1) * 128], zcm[:, g, :])

                # ---- Phase 2: xv in_proj + conv + V^T via XBAR + KV ----
                with (
                    tc.tile_pool(name="p1", bufs=2, space="PSUM") as p1,
                    tc.tile_pool(name="pkv", bufs=1, space="PSUM") as pkv,
                    tc.tile_pool(name="sbt", bufs=3) as sbt,
                    tc.tile_pool(name="cdgp2", bufs=2) as cdgp2,
                ):
                    kvp = pkv.tile([DSTATE, DIM], F32)
                    cdgv = []
                    for g in range(2):
                        c = cdgp2.tile([128, 9, 128], BF16, tag="cdg2")
                        nc.sync.dma_start(c[:], convdiag.ap()[:, g * 9:(g + 1) * 9, :])
                        cdgv.append(c)

                    def inproj_xv(n):
                        for g in range(2):
                            ps = p1.tile([128, 512], F32, tag="p1")
                            for k in range(2):
                                nc.tensor.matmul(ps[:], wxb[:, k, g * 128:(g + 1) * 128],
                                                 X[:, k, n * 512:(n + 1) * 512],
                                                 start=(k == 0), stop=(k == 1))
                            r0 = n * 8
                            dst = pads[g][:, r0 + 1:r0 + 9, 1:1 + W_]
                            src2 = ps[:].rearrange("p (r w) -> p r w", w=W_)
                            if g == 1:
                                nc.vector.tensor_copy(dst, src2)
                            else:
                                nc.scalar.copy(dst, src2)

                    def conv_v(n):
                        for g in range(2):
                            pc = pconv.tile([128, 512], F32, tag="pc")
                            r0 = n * 8
                            for ti in range(9):
                                dy, dx = ti // 3, ti % 3
                                nc.tensor.matmul(
                                    pc[:].rearrange("p (r w) -> p r w", w=W_),
                                    cdgv[g][:, ti, :],
                                    pads[g][:, r0 + dy:r0 + dy + 8, dx:dx + W_],
                                    start=(ti == 0), stop=(ti == 8))
                            nc.scalar.activation(Vg[g][:, n * 512:(n + 1) * 512], pc[:],
                                                 AF.Silu, bias=cb[:, g:g + 1])
                        if n % 2 == 1:
                            m = n // 2
                            for g in range(2):
                                eng = nc.sync if g == 0 else nc.scalar
                                eng.dma_start_transpose(
                                    Vt[:, 8 * m:8 * m + 8, g * 128:(g + 1) * 128],
                                    Vg[g][:, m * 1024:(m + 1) * 1024])

                    def ssd(m):
                        for i2 in range(4 * m, 4 * m + 4):
                            i = 2 * i2
                            vst = sbt.tile([128, 2, DIM], BF16, tag="vst")
                            nc.vector.tensor_mul(
                                vst[:].rearrange("p c (h q) -> p c h q", h=NHEADS),
                                Vt[:, i:i + 2, :].rearrange("p c (h q) -> p c h q", h=NHEADS),
                                dAsc[:, i:i + 2, :].unsqueeze(3).to_broadcast(
                                    [128, 2, NHEADS, HEAD_DIM]),
                            )
                            for j in range(2):
                                nc.tensor.matmul(kvp[:], Bt[:, i + j, :], vst[:, j, :],
                                                 start=(i + j == 0), stop=(i + j == LCH - 1))

                    for n in range(NCH + 2):
                        if n < NCH:
                            inproj_xv(n)
                        if 1 <= n < NCH + 1:
                            conv_v(n - 1)
                        if n >= 3 and (n - 3) % 2 == 0:
                            ssd((n - 3) // 2)
                    nc.scalar.copy(kv_sb[DSTATE:2 * DSTATE, :], kvp[:])

            # ---- Phase 3: y, D-skip, LN, gate, XBAR transpose, out_proj ----
            with (
                tc.tile_pool(name="pyp", bufs=3, space="PSUM") as pyp,
                tc.tile_pool(name="pout", bufs=2, space="PSUM") as pout,
                tc.tile_pool(name="s6", bufs=5) as s6,
                tc.tile_pool(name="s6g", bufs=2) as s6g,
                tc.tile_pool(name="yzp", bufs=1) as yzp,
            ):
                yz_all = yzp.tile([128, LCH, DIM], BF16, tag="yza")
                slabs = [None] * NCH
                outr = outd.ap().rearrange("(m k) l -> k m l", k=128)

                def emit_front(p):
                    ypr = pyp.tile([128, 2, DIM], F32, tag="ypr")
                    for j in range(2):
                        i = 2 * p + j
                        sl = slice(i * 128, (i + 1) * 128)
                        nc.tensor.matmul(ypr[:, j, :], BC[DSTATE:2 * DSTATE, sl],
                                         kv_sb[DSTATE:2 * DSTATE, :], start=True, stop=True)
                    # D-skip: y += V * D, added straight into PSUM on DVE
                    if generic_d:
                        vtmp = s6.tile([128, 2, DIM], F32, tag="vtmp")
                        nc.vector.tensor_mul(
                            vtmp[:], Vt[:, 2 * p:2 * p + 2, :],
                            dex_bc[:].unsqueeze(1).to_broadcast([128, 2, DIM]))
                        nc.vector.tensor_add(ypr[:], ypr[:], vtmp[:])
                    else:
                        nc.vector.tensor_add(ypr[:], ypr[:], Vt[:, 2 * p:2 * p + 2, :])

                    st = s6.tile([128, 2, 6], F32, tag="st")
                    for j in range(2):
                        nc.vector.bn_stats(st[:, j, :], ypr[:, j, :])
                    mv = s6.tile([128, 2, 2], F32, tag="mv")
                    for j in range(2):
                        nc.vector.bn_aggr(mv[:, j, :], st[:, j, :])
                    rstd = s6.tile([128, 2], F32, tag="rstd")
                    nc.scalar.activation(rstd[:], mv[:, :, 1], AF.Abs_reciprocal_sqrt,
                                         bias=eps_t[:])
                    negm = s6.tile([128, 2], F32, tag="negm")
                    nc.vector.tensor_scalar_mul(negm[:], mv[:, :, 0], -1.0)
                    nmr = s6.tile([128, 2], F32, tag="nmr")
                    nc.vector.tensor_mul(nmr[:], negm[:], rstd[:])
                    yn = s6.tile([128, 2, DIM], F32, tag="yn")
                    for j in range(2):
                        nc.scalar.activation(yn[:, j, :], ypr[:, j, :], AF.Identity,
                                             bias=nmr[:, j:j + 1], scale=rstd[:, j:j + 1])
                    if has_lnw:
                        for j in range(2):
                            nc.vector.tensor_mul(yn[:, j, :], yn[:, j, :], lnw_bc[:])
                    if has_lnb:
                        for j in range(2):
                            nc.vector.tensor_add(yn[:, j, :], yn[:, j, :], lnb_bc[:])
                    nc.gpsimd.tensor_mul(yz_all[:, 2 * p:2 * p + 2, :], yn[:],
                                         zt[:, 2 * p:2 * p + 2, :])

                def emit_tr(q):
                    # 4 chunks (512 l) per XBAR call; slab [cc, jc, g, l]
                    slab = s6g.tile([128, 4, 2, 128], BF16, tag="yzt")
                    nc.sync.dma_start_transpose(
                        slab[:],
                        yz_all[:, 4 * q:4 * q + 4, :].rearrange("a b c -> a (b c)"))
                    slabs[q] = slab

                def emit_out(q):
                    slab = slabs[q]
                    ob = s6g.tile([128, 2, 512], F32, tag="ob")
                    for mo in range(2):
                        po = pout.tile([128, 512], F32)
                        for k in range(2):
                            nc.tensor.matmul(po[:], wo[:, k, mo * 128:(mo + 1) * 128],
                                             slab[:, :, k, :], start=(k == 0), stop=(k == 1))
                        if mo == 0:
                            nc.vector.tensor_copy(ob[:, mo, :], po[:])
                        else:
                            nc.scalar.copy(ob[:, mo, :], po[:])
                    eng = nc.sync if q % 2 == 0 else nc.scalar
                    eng.dma_start(outr[:, :, q * 512:(q + 1) * 512], ob[:])

                NP = LCH // 2
                for p in range(NP + 6):
                    if p < NP:
                        emit_front(p)
                    if p >= 3 and (p - 3) % 2 == 0 and (p - 3) // 2 < NCH:
                        emit_tr((p - 3) // 2)
                    if p >= 5 and (p - 5) % 2 == 0 and (p - 5) // 2 < NCH:
                        emit_out((p - 5) // 2)


_CACHE = {}


def _prep(W_in, conv_w, conv_b, A_log, dt_bias, D, ln_w, ln_b, W_out):
    W_in = np.asarray(W_in, np.float32)
    conv_w = np.asarray(conv_w, np.float32)
    conv_b = np.asarray(conv_b, np.float32)
    A_log = np.asarray(A_log, np.float32)
    dt_bias = np.asarray(dt_bias, np.float32)
    D = np.asarray(D, np.float32)
    ln_w = np.asarray(ln_w, np.float32)
    ln_b = np.asarray(ln_b, np.float32)
    W_out = np.asarray(W_out, np.float32)

    WinT = np.ascontiguousarray(W_in.T)                       # [256, 648]
    w_xbcdt = np.ascontiguousarray(WinT[:, DIM:]).astype(BF)  # [256, 392]
    w_z = np.ascontiguousarray(WinT[:, :DIM]).astype(BF)      # [256, 256]
    w_out_t = np.ascontiguousarray(W_out.T).astype(BF)        # [256, 256]

    cd = np.zeros((128, 27, 128), np.float32)
    idx = np.arange(128)
    for g in range(3):
        for t in range(9):
            cd[idx, g * 9 + t, idx] = conv_w[g * 128 + idx, 0, t // 3, t % 3]

    cbm = np.zeros((128, 3), np.float32)
    for g in range(3):
        cbm[:, g] = conv_b[g * 128:(g + 1) * 128]

    flags = {
        "generic_d": not np.allclose(D, 1.0),
        "has_lnw": not np.allclose(ln_w, 1.0),
        "has_lnb": not np.allclose(ln_b, 0.0),
    }
    shared = dict(
        w_xbcdt=w_xbcdt, w_z=w_z, w_out=w_out_t, convdiag=cd.astype(BF),
        dtbias=dt_bias.reshape(NHEADS, 1).astype(np.float32),
        expa=np.exp(A_log).reshape(NHEADS, 1).astype(np.float32),
        convb=cbm, dexpand=np.repeat(D, HEAD_DIM).reshape(1, DIM).astype(np.float32),
        lnwv=ln_w, lnbv=ln_b,
    )
    return flags, shared


def _get_nc(flags, reps=1):
    key = (tuple(sorted(flags.items())), reps)
    if key not in _CACHE:
        _CACHE[key] = _build(flags, reps)
    return _CACHE[key]


def kernel(x, W_in, conv_w, conv_b, A_log, dt_bias, D, ln_w, ln_b, W_out,
           _trace=False):
    x = np.asarray(x, np.float32)
    flags, shared = _prep(W_in, conv_w, conv_b, A_log, dt_bias, D, ln_w, ln_b, W_out)
    nc = _get_nc(flags)
    xb = x.reshape(B_, DIM, L).astype(BF)
    in_maps = [dict(xb=np.ascontiguousarray(xb[b]), **shared) for b in range(B_)]
    res = run_bass_kernel_spmd(nc, in_maps, core_ids=list(range(NCORES)), trace=_trace)
    out = np.stack([res.results[b]["outb"] for b in range(B_)])
    out = out.reshape(B_, DIM, H_, W_)
    if _trace:
        return out, res
    return out


# revision 15
# speedup vs baseline: 1.1739x; 1.0194x over previous
# Bass/Tile TRN2 kernel for nn_BlqSSM (Mamba2-SSD-style block with depthwise
# 3x3 conv, non-causal linear attention, LayerNorm gate, out-projection).
#
# Sharding: data-parallel over batch — 8 batches on 8 NeuronCores, weights
# replicated, no collectives. Each core computes one full batch element.
#
# v2: bf16 datapath + XBAR DMA transposes. All PE-transpose matmuls from v1
# (V^T, B^T, dA^T, yz^T, D-skip) are replaced by dma_start_transpose (bf16
# only) or DVE adds, and activations/weights stream through the PE in bf16
# (1 cycle/row, half-size LDWEIGHTS). PSUM accumulation stays fp32.
#
# Per-core dataflow (L = H*W = 4096, channel-major = [C, L], L-major = [L, C]):
#   1. in_proj (PE, bf16): dt rows -> [16, L] (top 8 zero) -> XBAR transpose;
#      dA = Ln(Exp(dt + bias) + 1) * exp(A_log) in one contiguous ACT block
#      (exactly 3 act-table loads in the whole kernel). BC rows are written
#      into a zero-padded [128, 66, 66] conv buffer; z is computed
#      channel-major with stationary W_z slices and XBAR-transposed to
#      L-major during phase 2.
#   2. depthwise conv (PE, bf16): 9 taps as diag-weight matmuls accumulating
#      in PSUM over shifted views of the padded buffer; SiLU+bias fused into
#      the PSUM->SBUF evacuation on ACT, emitting bf16.
#   3. SSD (PE): per 512-col chunk, XBAR-transpose B and V to L-major;
#      V^T is scaled by dA on DVE (bf16); KV [64, 256] accumulates over all
#      32 chunks in one PSUM tile.
#   4. y = C @ KV per 128-row chunk; the D-skip term (y += V*D) is a DVE add
#      of the saved V^T directly into PSUM. LayerNorm (DVE bn_stats/bn_aggr
#      + ACT rsqrt) + gating with z; yz (bf16) is XBAR-transposed back to
#      channel-major and fed to out_proj (PE, bf16), output DMA'd as fp32.
import os
import sys

for _p in ("/opt/trn_rl_repo", os.path.expanduser("~/.axon_site/_ro/trn_rl_repo")):
    if os.path.isdir(_p) and _p not in sys.path:
        sys.path.insert(0, _p)

import numpy as np
import ml_dtypes

import concourse.bass as bass
import concourse.mybir as mybir
import concourse.tile as tile
from concourse import bacc
from concourse.bass_utils import run_bass_kernel_spmd

F32 = mybir.dt.float32
BF16 = mybir.dt.bfloat16
AF = mybir.ActivationFunctionType
ALU = mybir.AluOpType

DIM = 256
DSTATE = 64
NHEADS = 8
HEAD_DIM = DIM // NHEADS          # 32
CONV_DIM = DIM + 2 * DSTATE       # 384
D_IN_PROJ = 2 * DIM + 2 * DSTATE + NHEADS  # 648
LN_EPS = 1e-5
B_, H_, W_ = 8, 64, 64
L = H_ * W_                       # 4096
NCORES = 8
NCH = 8                           # 512-wide column chunks of L
LCH = 32                          # 128-wide column chunks of L
PADW = H_ + 2                     # 66

BF = ml_dtypes.bfloat16


def _build(flags, reps=1):
    generic_d = flags["generic_d"]
    has_lnw = flags["has_lnw"]
    has_lnb = flags["has_lnb"]

    nc = bacc.Bacc("TRN2", target_bir_lowering=False, debug=False, num_devices=NCORES)

    xd = nc.dram_tensor("xb", [DIM, L], BF16, kind="ExternalInput")
    w_xbcdt = nc.dram_tensor("w_xbcdt", [DIM, CONV_DIM + NHEADS], BF16, kind="ExternalInput")
    w_z = nc.dram_tensor("w_z", [DIM, DIM], BF16, kind="ExternalInput")
    w_out = nc.dram_tensor("w_out", [DIM, DIM], BF16, kind="ExternalInput")
    convdiag = nc.dram_tensor("convdiag", [128, 27, 128], BF16, kind="ExternalInput")
    dtbias = nc.dram_tensor("dtbias", [NHEADS, 1], F32, kind="ExternalInput")
    expa = nc.dram_tensor("expa", [NHEADS, 1], F32, kind="ExternalInput")
    convb = nc.dram_tensor("convb", [128, 3], F32, kind="ExternalInput")
    dexpand = nc.dram_tensor("dexpand", [1, DIM], F32, kind="ExternalInput")
    lnwv = nc.dram_tensor("lnwv", [DIM], F32, kind="ExternalInput")
    lnbv = nc.dram_tensor("lnbv", [DIM], F32, kind="ExternalInput")
    outd = nc.dram_tensor("outb", [DIM, L], F32, kind="ExternalOutput")

    with tile.TileContext(nc) as tc:
        _emit(nc, tc, locals(), generic_d, has_lnw, has_lnb, reps)
    nc.compile()
    return nc


def _emit(nc, tc, t, generic_d, has_lnw, has_lnb, reps=1):
    from contextlib import ExitStack

    xd, w_xbcdt, w_z, w_out = t["xd"], t["w_xbcdt"], t["w_z"], t["w_out"]
    convdiag, dtbias, expa = t["convdiag"], t["dtbias"], t["expa"]
    convb, dexpand, lnwv, lnbv, outd = t["convb"], t["dexpand"], t["lnwv"], t["lnbv"], t["outd"]

    with ExitStack() as ctx:
        const = ctx.enter_context(tc.tile_pool(name="const", bufs=1))
        big = ctx.enter_context(tc.tile_pool(name="big", bufs=1))

        # ---- weights + X, ordered so the first matmuls unblock earliest ----
        wxb = const.tile([128, 2, CONV_DIM + NHEADS], BF16)
        nc.sync.dma_start(wxb[:], w_xbcdt.ap().rearrange("(t k) m -> k t m", k=128))
        X = big.tile([128, 2, L], BF16)
        xr = xd.ap().rearrange("(t k) l -> k t l", k=128)
        for c in range(4):
            for k in range(2):
                eng = nc.sync if k == 0 else nc.scalar
                eng.dma_start(X[:, k, c * 1024:(c + 1) * 1024], xr[:, k, c * 1024:(c + 1) * 1024])
        dtb_t = const.tile([128, NHEADS], F32)
        nc.sync.dma_start(dtb_t[:], dtbias.ap().rearrange("a b -> b a").partition_broadcast(128))
        ea_t = const.tile([128, NHEADS], F32)
        nc.sync.dma_start(ea_t[:], expa.ap().rearrange("a b -> b a").partition_broadcast(128))
        cb = const.tile([128, 3], F32)
        nc.sync.dma_start(cb[:], convb.ap())
        eps_t = const.tile([128, 1], F32)
        nc.vector.memset(eps_t[:], LN_EPS)
        wz = const.tile([128, 2, DIM], BF16)
        nc.scalar.dma_start(wz[:], w_z.ap().rearrange("(t k) m -> k t m", k=128))
        wo = const.tile([128, 2, DIM], BF16)
        nc.scalar.dma_start(wo[:], w_out.ap().rearrange("(t k) m -> k t m", k=128))
        if generic_d:
            dex_bc = const.tile([128, DIM], F32)
            nc.sync.dma_start(dex_bc[:], dexpand.ap().partition_broadcast(128))
        if has_lnw:
            lnw_bc = const.tile([128, DIM], F32)
            nc.sync.dma_start(lnw_bc[:], lnwv.ap().unsqueeze(0).partition_broadcast(128))
        if has_lnb:
            lnb_bc = const.tile([128, DIM], F32)
            nc.sync.dma_start(lnb_bc[:], lnbv.ap().unsqueeze(0).partition_broadcast(128))

        # ---- persistent activations ----
        pads = [big.tile([128, PADW, PADW], BF16, name=f"pad{g}") for g in range(3)]
        V0 = big.tile([128, L], BF16)
        V1 = big.tile([128, L], BF16)
        Vg = [V0, V1]
        Vt = big.tile([128, LCH, DIM], BF16)      # V^T, L-major, per-chunk
        BC = big.tile([128, L], BF16)             # B at parts 0:64, C at 64:128
        Bt = big.tile([128, LCH, DSTATE], BF16)   # B^T, L-major
        zt = big.tile([128, LCH, DIM], BF16)      # z, L-major
        dtraw = big.tile([16, L], BF16)
        dtt = big.tile([128, LCH, 16], BF16)
        dAsc = big.tile([128, LCH, NHEADS], BF16)
        kv_sb = big.tile([128, DIM], BF16)        # KV parked at partitions 64:128

        nc.gpsimd.memset(dtraw[:], 0.0)

        for _rep in range(reps):
            # border-only zeroing of the conv pads (interior fully overwritten)
            for g in range(3):
                p = pads[g][:]
                nc.gpsimd.memset(p[:, 0:1, :], 0.0)
                nc.gpsimd.memset(p[:, PADW - 1:PADW, :], 0.0)
                nc.gpsimd.memset(p[:, 1:PADW - 1, 0:1], 0.0)
                nc.gpsimd.memset(p[:, 1:PADW - 1, PADW - 1:PADW], 0.0)

            with (
                tc.tile_pool(name="pconv", bufs=2, space="PSUM") as pconv,
                tc.tile_pool(name="p1a", bufs=2, space="PSUM") as p1a,
                tc.tile_pool(name="pdt", bufs=2, space="PSUM") as pdt,
                tc.tile_pool(name="pkv", bufs=1, space="PSUM") as pkv,
                tc.tile_pool(name="daf", bufs=1) as daf_pool,
                tc.tile_pool(name="cdgp", bufs=1) as cdgp,
                tc.tile_pool(name="zcmp", bufs=1) as zcmp,
                tc.tile_pool(name="sbt", bufs=3) as sbt,
            ):
                # ---- Phase 1: dt in_proj + dA chain; BC + z in_proj; conv(BC) ----
                if True:
                    cdg2 = cdgp.tile([128, 9, 128], BF16, tag="cdg")
                    nc.sync.dma_start(cdg2[:], convdiag.ap()[:, 18:27, :])
                    zcm = zcmp.tile([128, 2, L], BF16, tag="zcm")

                    for n in range(NCH):
                        ps = pdt.tile([NHEADS, 512], F32)
                        for k in range(2):
                            nc.tensor.matmul(ps[:], wxb[:, k, CONV_DIM:],
                                             X[:, k, n * 512:(n + 1) * 512],
                                             start=(k == 0), stop=(k == 1))
                        nc.vector.tensor_copy(dtraw[0:NHEADS, n * 512:(n + 1) * 512], ps[:])
                    nc.sync.dma_start_transpose(dtt[:], dtraw[:])
                    # dA = Ln(Exp(dt + bias) + 1) * exp(A_log), one contiguous
                    # ACT block so only one Exp/Ln table set load happens.
                    dAf = daf_pool.tile([128, LCH, NHEADS], F32, tag="dAf")
                    nc.vector.tensor_add(
                        dAf[:], dtt[:, :, 0:NHEADS],
                        dtb_t[:].unsqueeze(1).to_broadcast([128, LCH, NHEADS]))
                    nc.scalar.activation(dAf[:], dAf[:], AF.Exp)
                    nc.scalar.activation(dAf[:], dAf[:], AF.Ln, bias=1.0)
                    nc.vector.tensor_mul(
                        dAsc[:], dAf[:],
                        ea_t[:].unsqueeze(1).to_broadcast([128, LCH, NHEADS]))

                    # BC in_proj -> pad buffer
                    for n in range(NCH):
                        ps = p1a.tile([128, 512], F32)
                        for k in range(2):
                            nc.tensor.matmul(ps[:], wxb[:, k, 2 * 128:3 * 128],
                                             X[:, k, n * 512:(n + 1) * 512],
                                             start=(k == 0), stop=(k == 1))
                        r0 = n * 8
                        nc.vector.tensor_copy(pads[2][:, r0 + 1:r0 + 9, 1:1 + W_],
                                              ps[:].rearrange("p (r w) -> p r w", w=W_))
                    # z in_proj, channel-major (stationary W_z slices)
                    for g in range(2):
                        for n in range(NCH):
                            ps = p1a.tile([128, 512], F32)
                            for k in range(2):
                                nc.tensor.matmul(ps[:], wz[:, k, g * 128:(g + 1) * 128],
                                                 X[:, k, n * 512:(n + 1) * 512],
                                                 start=(k == 0), stop=(k == 1))
                            if n % 2 == 0:
                                nc.vector.tensor_copy(zcm[:, g, n * 512:(n + 1) * 512], ps[:])
                            else:
                                nc.scalar.copy(zcm[:, g, n * 512:(n + 1) * 512], ps[:])
                    # conv(BC) + SiLU evacuation
                    for n in range(NCH):
                        pc = pconv.tile([128, 512], F32, tag="pc")
                        r0 = n * 8
                        for ti in range(9):
                            dy, dx = ti // 3, ti % 3
                            nc.tensor.matmul(
                                pc[:].rearrange("p (r w) -> p r w", w=W_),
                                cdg2[:, ti, :],
                                pads[2][:, r0 + dy:r0 + dy + 8, dx:dx + W_],
                                start=(ti == 0), stop=(ti == 8))
                        nc.scalar.activation(BC[:, n * 512:(n + 1) * 512], pc[:],
                                             AF.Silu, bias=cb[:, 2:3])
                    # B^T and z^T via XBAR (overlap phase 2 compute)
                    nc.sync.dma_start_transpose(Bt[:], BC[0:DSTATE, :])
                    for g in range(2):
                        nc.scalar.dma_start_transpose(
                            zt[:, :, g * 128:(g + 1) * 128], zcm[:, g, :])

                # ---- Phase 2: xv in_proj + conv + V^T via XBAR + KV ----
                with (
                    tc.tile_pool(name="cdgp2", bufs=2) as cdgp2,
                ):
                    kvp = pkv.tile([DSTATE, DIM], F32)
                    cdgv = []
                    for g in range(2):
                        c = cdgp2.tile([128, 9, 128], BF16, tag="cdg2")
                        nc.sync.dma_start(c[:], convdiag.ap()[:, g * 9:(g + 1) * 9, :])
                        cdgv.append(c)

                    def inproj_xv(n):
                        for g in range(2):
                            ps = p1a.tile([128, 512], F32)
                            for k in range(2):
                                nc.tensor.matmul(ps[:], wxb[:, k, g * 128:(g + 1) * 128],
                                                 X[:, k, n * 512:(n + 1) * 512],
                                                 start=(k == 0), stop=(k == 1))
                            r0 = n * 8
                            dst = pads[g][:, r0 + 1:r0 + 9, 1:1 + W_]
                            src2 = ps[:].rearrange("p (r w) -> p r w", w=W_)
                            if g == 1:
                                nc.vector.tensor_copy(dst, src2)
                            else:
                                nc.scalar.copy(dst, src2)

                    def conv_v(n):
                        for g in range(2):
                            pc = pconv.tile([128, 512], F32, tag="pc")
                            r0 = n * 8
                            for ti in range(9):
                                dy, dx = ti // 3, ti % 3
                                nc.tensor.matmul(
                                    pc[:].rearrange("p (r w) -> p r w", w=W_),
                                    cdgv[g][:, ti, :],
                                    pads[g][:, r0 + dy:r0 + dy + 8, dx:dx + W_],
                                    start=(ti == 0), stop=(ti == 8))
                            nc.scalar.activation(Vg[g][:, n * 512:(n + 1) * 512], pc[:],
                                                 AF.Silu, bias=cb[:, g:g + 1])
                        if n % 2 == 1:
                            m = n // 2
                            for g in range(2):
                                eng = nc.sync if g == 0 else nc.scalar
                                eng.dma_start_transpose(
                                    Vt[:, 8 * m:8 * m + 8, g * 128:(g + 1) * 128],
                                    Vg[g][:, m * 1024:(m + 1) * 1024])

                    def ssd(m):
                        for i2 in range(4 * m, 4 * m + 4):
                            i = 2 * i2
                            vst = sbt.tile([128, 2, DIM], BF16, tag="vst")
                            nc.vector.tensor_mul(
                                vst[:].rearrange("p c (h q) -> p c h q", h=NHEADS),
                                Vt[:, i:i + 2, :].rearrange("p c (h q) -> p c h q", h=NHEADS),
                                dAsc[:, i:i + 2, :].unsqueeze(3).to_broadcast(
                                    [128, 2, NHEADS, HEAD_DIM]),
                            )
                            for j in range(2):
                                nc.tensor.matmul(kvp[:], Bt[:, i + j, :], vst[:, j, :],
                                                 start=(i + j == 0), stop=(i + j == LCH - 1))

                    for n in range(NCH + 2):
                        if n < NCH:
                            inproj_xv(n)
                        if 1 <= n < NCH + 1:
                            conv_v(n - 1)
                        if n >= 3 and (n - 3) % 2 == 0:
                            ssd((n - 3) // 2)
                    nc.scalar.copy(kv_sb[DSTATE:2 * DSTATE, :], kvp[:])

            # ---- Phase 3: y, D-skip, LN, gate, XBAR transpose, out_proj ----
            # Quad-batched: 4 L-chunks per iteration; ypr spans 2 PSUM banks.
            with (
                tc.tile_pool(name="pyp", bufs=2, space="PSUM") as pyp,
                tc.tile_pool(name="pout", bufs=2, space="PSUM") as pout,
                tc.tile_pool(name="s6", bufs=3) as s6,
                tc.tile_pool(name="s6g", bufs=2) as s6g,
                tc.tile_pool(name="yzp", bufs=1) as yzp,
            ):
                yz_all = yzp.tile([128, LCH, DIM], BF16, tag="yza")
                slabs = [None] * NCH
                outr = outd.ap().rearrange("(m k) l -> k m l", k=128)

                def emit_front(q):
                    ypr = pyp.tile([128, 4, DIM], F32, tag="ypr")
                    for j in range(4):
                        i = 4 * q + j
                        sl = slice(i * 128, (i + 1) * 128)
                        nc.tensor.matmul(ypr[:, j, :], BC[DSTATE:2 * DSTATE, sl],
                                         kv_sb[DSTATE:2 * DSTATE, :], start=True, stop=True)
                    # D-skip: y += V * D, added straight into PSUM on DVE
                    if generic_d:
                        vtmp = s6.tile([128, 4, DIM], F32, tag="vtmp")
                        nc.vector.tensor_mul(
                            vtmp[:], Vt[:, 4 * q:4 * q + 4, :],
                            dex_bc[:].unsqueeze(1).to_broadcast([128, 4, DIM]))
                        nc.vector.tensor_add(ypr[:], ypr[:], vtmp[:])
                    else:
                        nc.vector.tensor_add(ypr[:], ypr[:], Vt[:, 4 * q:4 * q + 4, :])

                    st = s6.tile([128, 4, 6], F32, tag="st")
                    for j in range(4):
                        nc.vector.bn_stats(st[:, j, :], ypr[:, j, :])
                    mv = s6.tile([128, 4, 2], F32, tag="mv")
                    for j in range(4):
                        nc.vector.bn_aggr(mv[:, j, :], st[:, j, :])
                    rstd = s6.tile([128, 4], F32, tag="rstd")
                    nc.scalar.activation(rstd[:], mv[:, :, 1], AF.Abs_reciprocal_sqrt,
                                         bias=eps_t[:])
                    negm = s6.tile([128, 4], F32, tag="negm")
                    nc.gpsimd.tensor_scalar_mul(negm[:], mv[:, :, 0], -1.0)
                    nmr = s6.tile([128, 4], F32, tag="nmr")
                    nc.gpsimd.tensor_mul(nmr[:], negm[:], rstd[:])
                    yn = s6.tile([128, 4, DIM], F32, tag="yn")
                    for j in range(4):
                        nc.scalar.activation(yn[:, j, :], ypr[:, j, :], AF.Identity,
                                             bias=nmr[:, j:j + 1], scale=rstd[:, j:j + 1])
                    if has_lnw:
                        nc.vector.tensor_mul(
                            yn[:], yn[:],
                            lnw_bc[:].unsqueeze(1).to_broadcast([128, 4, DIM]))
                    if has_lnb:
                        nc.vector.tensor_add(
                            yn[:], yn[:],
                            lnb_bc[:].unsqueeze(1).to_broadcast([128, 4, DIM]))
                    nc.gpsimd.tensor_mul(yz_all[:, 4 * q:4 * q + 4, :], yn[:],
                                         zt[:, 4 * q:4 * q + 4, :])

                def emit_tr(q):
                    # 4 chunks (512 l) per XBAR call; slab [cc, jc, g, l]
                    slab = s6g.tile([128, 4, 2, 128], BF16, tag="yzt")
                    nc.sync.dma_start_transpose(
                        slab[:],
                        yz_all[:, 4 * q:4 * q + 4, :].rearrange("a b c -> a (b c)"))
                    slabs[q] = slab

                def emit_out(q):
                    slab = slabs[q]
                    ob = s6g.tile([128, 2, 512], F32, tag="ob")
                    for mo in range(2):
                        po = pout.tile([128, 512], F32)
                        for k in range(2):
                            nc.tensor.matmul(po[:], wo[:, k, mo * 128:(mo + 1) * 128],
                                             slab[:, :, k, :], start=(k == 0), stop=(k == 1))
                        if mo == 0:
                            nc.vector.tensor_copy(ob[:, mo, :], po[:])
                        else:
                            nc.scalar.copy(ob[:, mo, :], po[:])
                    eng = nc.sync if q % 2 == 0 else nc.scalar
                    eng.dma_start(outr[:, :, q * 512:(q + 1) * 512], ob[:])

                for q in range(NCH + 2):
                    if q < NCH:
                        emit_front(q)
                    if q >= 1 and q - 1 < NCH:
                        emit_tr(q - 1)
                    if q >= 2 and q - 2 < NCH:
                        emit_out(q - 2)


_CACHE = {}


def _prep(W_in, conv_w, conv_b, A_log, dt_bias, D, ln_w, ln_b, W_out):
    W_in = np.asarray(W_in, np.float32)
    conv_w = np.asarray(conv_w, np.float32)
    conv_b = np.asarray(conv_b, np.float32)
    A_log = np.asarray(A_log, np.float32)
    dt_bias = np.asarray(dt_bias, np.float32)
    D = np.asarray(D, np.float32)
    ln_w = np.asarray(ln_w, np.float32)
    ln_b = np.asarray(ln_b, np.float32)
    W_out = np.asarray(W_out, np.float32)

    WinT = np.ascontiguousarray(W_in.T)                       # [256, 648]
    w_xbcdt = np.ascontiguousarray(WinT[:, DIM:]).astype(BF)  # [256, 392]
    w_z = np.ascontiguousarray(WinT[:, :DIM]).astype(BF)      # [256, 256]
    w_out_t = np.ascontiguousarray(W_out.T).astype(BF)        # [256, 256]

    cd = np.zeros((128, 27, 128), np.float32)
    idx = np.arange(128)
    for g in range(3):
        for t in range(9):
            cd[idx, g * 9 + t, idx] = conv_w[g * 128 + idx, 0, t // 3, t % 3]

    cbm = np.zeros((128, 3), np.float32)
    for g in range(3):
        cbm[:, g] = conv_b[g * 128:(g + 1) * 128]

    flags = {
        "generic_d": not np.allclose(D, 1.0),
        "has_lnw": not np.allclose(ln_w, 1.0),
        "has_lnb": not np.allclose(ln_b, 0.0),
    }
    shared = dict(
        w_xbcdt=w_xbcdt, w_z=w_z, w_out=w_out_t, convdiag=cd.astype(BF),
        dtbias=dt_bias.reshape(NHEADS, 1).astype(np.float32),
        expa=np.exp(A_log).reshape(NHEADS, 1).astype(np.float32),
        convb=cbm, dexpand=np.repeat(D, HEAD_DIM).reshape(1, DIM).astype(np.float32),
        lnwv=ln_w, lnbv=ln_b,
    )
    return flags, shared


def _get_nc(flags, reps=1):
    key = (tuple(sorted(flags.items())), reps)
    if key not in _CACHE:
        _CACHE[key] = _build(flags, reps)
    return _CACHE[key]


def kernel(x, W_in, conv_w, conv_b, A_log, dt_bias, D, ln_w, ln_b, W_out,
           _trace=False):
    x = np.asarray(x, np.float32)
    flags, shared = _prep(W_in, conv_w, conv_b, A_log, dt_bias, D, ln_w, ln_b, W_out)
    nc = _get_nc(flags)
    xb = x.reshape(B_, DIM, L).astype(BF)
    in_maps = [dict(xb=np.ascontiguousarray(xb[b]), **shared) for b in range(B_)]
    res = run_bass_kernel_spmd(nc, in_maps, core_ids=list(range(NCORES)), trace=_trace)
    out = np.stack([res.results[b]["outb"] for b in range(B_)])
    out = out.reshape(B_, DIM, H_, W_)
    if _trace:
        return out, res
    return out


# revision 18
# speedup vs baseline: 1.2695x; 1.0814x over previous
# Bass/Tile TRN2 kernel for nn_BlqSSM (Mamba2-SSD-style block with depthwise
# 3x3 conv, non-causal linear attention, LayerNorm gate, out-projection).
#
# Sharding: data-parallel over batch — 8 batches on 8 NeuronCores, weights
# replicated, no collectives. Each core computes one full batch element.
#
# v2: bf16 datapath + XBAR DMA transposes. All PE-transpose matmuls from v1
# (V^T, B^T, dA^T, yz^T, D-skip) are replaced by dma_start_transpose (bf16
# only) or DVE adds, and activations/weights stream through the PE in bf16
# (1 cycle/row, half-size LDWEIGHTS). PSUM accumulation stays fp32.
#
# Per-core dataflow (L = H*W = 4096, channel-major = [C, L], L-major = [L, C]):
#   1. in_proj (PE, bf16): dt rows -> [16, L] (top 8 zero) -> XBAR transpose;
#      dA = Ln(Exp(dt + bias) + 1) * exp(A_log) in one contiguous ACT block
#      (exactly 3 act-table loads in the whole kernel). BC rows are written
#      into a zero-padded [128, 66, 66] conv buffer; z is computed
#      channel-major with stationary W_z slices and XBAR-transposed to
#      L-major during phase 2.
#   2. depthwise conv (PE, bf16): 9 taps as diag-weight matmuls accumulating
#      in PSUM over shifted views of the padded buffer; SiLU+bias fused into
#      the PSUM->SBUF evacuation on ACT, emitting bf16.
#   3. SSD (PE): per 512-col chunk, XBAR-transpose B and V to L-major;
#      V^T is scaled by dA on DVE (bf16); KV [64, 256] accumulates over all
#      32 chunks in one PSUM tile.
#   4. y = C @ KV per 128-row chunk; the D-skip term (y += V*D) is a DVE add
#      of the saved V^T directly into PSUM. LayerNorm (DVE bn_stats/bn_aggr
#      + ACT rsqrt) + gating with z; yz (bf16) is XBAR-transposed back to
#      channel-major and fed to out_proj (PE, bf16), output DMA'd as fp32.
import os
import sys

for _p in ("/opt/trn_rl_repo", os.path.expanduser("~/.axon_site/_ro/trn_rl_repo")):
    if os.path.isdir(_p) and _p not in sys.path:
        sys.path.insert(0, _p)

import numpy as np
import ml_dtypes

import concourse.bass as bass
import concourse.mybir as mybir
import concourse.tile as tile
from concourse import bacc
from concourse.bass_utils import run_bass_kernel_spmd

F32 = mybir.dt.float32
BF16 = mybir.dt.bfloat16
AF = mybir.ActivationFunctionType
ALU = mybir.AluOpType

DIM = 256
DSTATE = 64
NHEADS = 8
HEAD_DIM = DIM // NHEADS          # 32
CONV_DIM = DIM + 2 * DSTATE       # 384
D_IN_PROJ = 2 * DIM + 2 * DSTATE + NHEADS  # 648
LN_EPS = 1e-5
B_, H_, W_ = 8, 64, 64
L = H_ * W_                       # 4096
NCORES = 8
NCH = 8                           # 512-wide column chunks of L
LCH = 32                          # 128-wide column chunks of L
PADW = H_ + 2                     # 66

BF = ml_dtypes.bfloat16


def _build(flags, reps=1):
    generic_d = flags["generic_d"]
    has_lnw = flags["has_lnw"]
    has_lnb = flags["has_lnb"]

    nc = bacc.Bacc("TRN2", target_bir_lowering=False, debug=False, num_devices=NCORES)

    xd = nc.dram_tensor("xb", [DIM, L], BF16, kind="ExternalInput")
    w_xbcdt = nc.dram_tensor("w_xbcdt", [DIM, CONV_DIM + NHEADS], BF16, kind="ExternalInput")
    w_z = nc.dram_tensor("w_z", [DIM, DIM], BF16, kind="ExternalInput")
    w_out = nc.dram_tensor("w_out", [DIM, DIM], BF16, kind="ExternalInput")
    convdiag = nc.dram_tensor("convdiag", [128, 27, 128], BF16, kind="ExternalInput")
    dtbias = nc.dram_tensor("dtbias", [NHEADS, 1], F32, kind="ExternalInput")
    expa = nc.dram_tensor("expa", [NHEADS, 1], F32, kind="ExternalInput")
    convb = nc.dram_tensor("convb", [128, 3], F32, kind="ExternalInput")
    dexpand = nc.dram_tensor("dexpand", [1, DIM], F32, kind="ExternalInput")
    lnwv = nc.dram_tensor("lnwv", [DIM], F32, kind="ExternalInput")
    lnbv = nc.dram_tensor("lnbv", [DIM], F32, kind="ExternalInput")
    outd = nc.dram_tensor("outb", [DIM, L], F32, kind="ExternalOutput")

    with tile.TileContext(nc) as tc:
        _emit(nc, tc, locals(), generic_d, has_lnw, has_lnb, reps)
    nc.compile()
    return nc


def _emit(nc, tc, t, generic_d, has_lnw, has_lnb, reps=1):
    from contextlib import ExitStack

    xd, w_xbcdt, w_z, w_out = t["xd"], t["w_xbcdt"], t["w_z"], t["w_out"]
    convdiag, dtbias, expa = t["convdiag"], t["dtbias"], t["expa"]
    convb, dexpand, lnwv, lnbv, outd = t["convb"], t["dexpand"], t["lnwv"], t["lnbv"], t["outd"]

    with ExitStack() as ctx:
        const = ctx.enter_context(tc.tile_pool(name="const", bufs=1))
        big = ctx.enter_context(tc.tile_pool(name="big", bufs=1))

        # ---- weights + X, ordered so the first matmuls unblock earliest ----
        wxb = const.tile([128, 2, CONV_DIM + NHEADS], BF16)
        nc.sync.dma_start(wxb[:], w_xbcdt.ap().rearrange("(t k) m -> k t m", k=128))
        X = big.tile([128, 2, L], BF16)
        xr = xd.ap().rearrange("(t k) l -> k t l", k=128)
        for c in range(4):
            for k in range(2):
                eng = nc.sync if k == 0 else nc.scalar
                eng.dma_start(X[:, k, c * 1024:(c + 1) * 1024], xr[:, k, c * 1024:(c + 1) * 1024])
        dtb_t = const.tile([128, NHEADS], F32)
        nc.sync.dma_start(dtb_t[:], dtbias.ap().rearrange("a b -> b a").partition_broadcast(128))
        ea_t = const.tile([128, NHEADS], F32)
        nc.sync.dma_start(ea_t[:], expa.ap().rearrange("a b -> b a").partition_broadcast(128))
        cb = const.tile([128, 3], F32)
        nc.sync.dma_start(cb[:], convb.ap())
        eps_t = const.tile([128, 1], F32)
        nc.vector.memset(eps_t[:], LN_EPS)
        wz = const.tile([128, 2, DIM], BF16)
        nc.scalar.dma_start(wz[:], w_z.ap().rearrange("(t k) m -> k t m", k=128))
        wo = const.tile([128, 2, DIM], BF16)
        nc.scalar.dma_start(wo[:], w_out.ap().rearrange("(t k) m -> k t m", k=128))
        if generic_d:
            dex_bc = const.tile([128, DIM], F32)
            nc.sync.dma_start(dex_bc[:], dexpand.ap().partition_broadcast(128))
        if has_lnw:
            lnw_bc = const.tile([128, DIM], F32)
            nc.sync.dma_start(lnw_bc[:], lnwv.ap().unsqueeze(0).partition_broadcast(128))
        if has_lnb:
            lnb_bc = const.tile([128, DIM], F32)
            nc.sync.dma_start(lnb_bc[:], lnbv.ap().unsqueeze(0).partition_broadcast(128))

        # ---- persistent activations ----
        pads = [big.tile([128, PADW, PADW], BF16, name=f"pad{g}") for g in range(3)]
        V0 = big.tile([128, L], BF16)
        V1 = big.tile([128, L], BF16)
        Vg = [V0, V1]
        Vt = big.tile([128, LCH, DIM], BF16)      # V^T, L-major, per-chunk
        BC = big.tile([128, L], BF16)             # B at parts 0:64, C at 64:128
        Bt = big.tile([128, LCH, DSTATE], BF16)   # B^T, L-major
        zt = big.tile([128, LCH, DIM], BF16)      # z, L-major
        dtraw = big.tile([16, L], BF16)
        dtt = big.tile([128, LCH, 16], BF16)
        dAsc = big.tile([128, LCH, NHEADS], BF16)
        kv_sb = big.tile([128, DIM], BF16)        # KV parked at partitions 64:128

        nc.gpsimd.memset(dtraw[:], 0.0)

        for _rep in range(reps):
            # border-only zeroing of the conv pads (interior fully overwritten)
            for g in range(3):
                p = pads[g][:]
                nc.gpsimd.memset(p[:, 0:1, :], 0.0)
                nc.gpsimd.memset(p[:, PADW - 1:PADW, :], 0.0)
                nc.gpsimd.memset(p[:, 1:PADW - 1, 0:1], 0.0)
                nc.gpsimd.memset(p[:, 1:PADW - 1, PADW - 1:PADW], 0.0)

            with (
                tc.tile_pool(name="pconv", bufs=2, space="PSUM") as pconv,
                tc.tile_pool(name="p1a", bufs=2, space="PSUM") as p1a,
                tc.tile_pool(name="pdt", bufs=2, space="PSUM") as pdt,
                tc.tile_pool(name="pkv", bufs=1, space="PSUM") as pkv,
                tc.tile_pool(name="daf", bufs=1) as daf_pool,
                tc.tile_pool(name="cdgp", bufs=1) as cdgp,
                tc.tile_pool(name="zcmp", bufs=1) as zcmp,
                tc.tile_pool(name="sbt", bufs=3) as sbt,
            ):
                # ---- Phase 1: dt in_proj + dA chain; BC + z in_proj; conv(BC) ----
                if True:
                    cdg2 = cdgp.tile([128, 9, 128], BF16, tag="cdg")
                    nc.sync.dma_start(cdg2[:], convdiag.ap()[:, 18:27, :])
                    zcm = zcmp.tile([128, 2, L], BF16, tag="zcm")

                    for n in range(NCH):
                        ps = pdt.tile([NHEADS, 512], F32)
                        for k in range(2):
                            nc.tensor.matmul(ps[:], wxb[:, k, CONV_DIM:],
                                             X[:, k, n * 512:(n + 1) * 512],
                                             start=(k == 0), stop=(k == 1))
                        nc.vector.tensor_copy(dtraw[0:NHEADS, n * 512:(n + 1) * 512], ps[:])
                    nc.sync.dma_start_transpose(dtt[:], dtraw[:])
                    # dA = Ln(Exp(dt + bias) + 1) * exp(A_log), one contiguous
                    # ACT block so only one Exp/Ln table set load happens.
                    dAf = daf_pool.tile([128, LCH, NHEADS], F32, tag="dAf")
                    nc.vector.tensor_add(
                        dAf[:], dtt[:, :, 0:NHEADS],
                        dtb_t[:].unsqueeze(1).to_broadcast([128, LCH, NHEADS]))
                    nc.scalar.activation(dAf[:], dAf[:], AF.Exp)
                    nc.scalar.activation(dAf[:], dAf[:], AF.Ln, bias=1.0)
                    nc.vector.tensor_mul(
                        dAsc[:], dAf[:],
                        ea_t[:].unsqueeze(1).to_broadcast([128, LCH, NHEADS]))

                    # BC in_proj -> pad buffer
                    for n in range(NCH):
                        ps = p1a.tile([128, 512], F32)
                        for k in range(2):
                            nc.tensor.matmul(ps[:], wxb[:, k, 2 * 128:3 * 128],
                                             X[:, k, n * 512:(n + 1) * 512],
                                             start=(k == 0), stop=(k == 1))
                        r0 = n * 8
                        nc.vector.tensor_copy(pads[2][:, r0 + 1:r0 + 9, 1:1 + W_],
                                              ps[:].rearrange("p (r w) -> p r w", w=W_))
                    # z in_proj, channel-major (stationary W_z slices)
                    for g in range(2):
                        for n in range(NCH):
                            ps = p1a.tile([128, 512], F32)
                            for k in range(2):
                                nc.tensor.matmul(ps[:], wz[:, k, g * 128:(g + 1) * 128],
                                                 X[:, k, n * 512:(n + 1) * 512],
                                                 start=(k == 0), stop=(k == 1))
                            if n % 2 == 0:
                                nc.vector.tensor_copy(zcm[:, g, n * 512:(n + 1) * 512], ps[:])
                            else:
                                nc.scalar.copy(zcm[:, g, n * 512:(n + 1) * 512], ps[:])
                    # conv(BC) + SiLU evacuation
                    for n in range(NCH):
                        pc = pconv.tile([128, 512], F32, tag="pc")
                        r0 = n * 8
                        for ti in range(9):
                            dy, dx = ti // 3, ti % 3
                            nc.tensor.matmul(
                                pc[:].rearrange("p (r w) -> p r w", w=W_),
                                cdg2[:, ti, :],
                                pads[2][:, r0 + dy:r0 + dy + 8, dx:dx + W_],
                                start=(ti == 0), stop=(ti == 8))
                        nc.scalar.activation(BC[:, n * 512:(n + 1) * 512], pc[:],
                                             AF.Silu, bias=cb[:, 2:3])
                    # B^T and z^T via XBAR (overlap phase 2 compute)
                    nc.sync.dma_start_transpose(Bt[:], BC[0:DSTATE, :])
                    for g in range(2):
                        nc.scalar.dma_start_transpose(
                            zt[:, :, g * 128:(g + 1) * 128], zcm[:, g, :])

                # ---- Phase 2: xv in_proj + conv + V^T via XBAR + KV ----
                with (
                    tc.tile_pool(name="cdgp2", bufs=2) as cdgp2,
                ):
                    kvp = pkv.tile([DSTATE, DIM], F32)
                    cdgv = []
                    for g in range(2):
                        c = cdgp2.tile([128, 9, 128], BF16, tag="cdg2")
                        nc.sync.dma_start(c[:], convdiag.ap()[:, g * 9:(g + 1) * 9, :])
                        cdgv.append(c)

                    def inproj_xv(n):
                        for g in range(2):
                            ps = p1a.tile([128, 512], F32)
                            for k in range(2):
                                nc.tensor.matmul(ps[:], wxb[:, k, g * 128:(g + 1) * 128],
                                                 X[:, k, n * 512:(n + 1) * 512],
                                                 start=(k == 0), stop=(k == 1))
                            r0 = n * 8
                            dst = pads[g][:, r0 + 1:r0 + 9, 1:1 + W_]
                            src2 = ps[:].rearrange("p (r w) -> p r w", w=W_)
                            if g == 1:
                                nc.vector.tensor_copy(dst, src2)
                            else:
                                nc.scalar.copy(dst, src2)

                    def conv_v(n):
                        for g in range(2):
                            pc = pconv.tile([128, 512], F32, tag="pc")
                            r0 = n * 8
                            for ti in range(9):
                                dy, dx = ti // 3, ti % 3
                                nc.tensor.matmul(
                                    pc[:].rearrange("p (r w) -> p r w", w=W_),
                                    cdgv[g][:, ti, :],
                                    pads[g][:, r0 + dy:r0 + dy + 8, dx:dx + W_],
                                    start=(ti == 0), stop=(ti == 8))
                            nc.scalar.activation(Vg[g][:, n * 512:(n + 1) * 512], pc[:],
                                                 AF.Silu, bias=cb[:, g:g + 1])
                        for g in range(2):
                            eng = nc.sync if g == 0 else nc.scalar
                            eng.dma_start_transpose(
                                Vt[:, 4 * n:4 * n + 4, g * 128:(g + 1) * 128],
                                Vg[g][:, n * 512:(n + 1) * 512])

                    def ssd(n):
                        for i in (4 * n, 4 * n + 2):
                            vst = sbt.tile([128, 2, DIM], BF16, tag="vst")
                            nc.vector.tensor_mul(
                                vst[:].rearrange("p c (h q) -> p c h q", h=NHEADS),
                                Vt[:, i:i + 2, :].rearrange("p c (h q) -> p c h q", h=NHEADS),
                                dAsc[:, i:i + 2, :].unsqueeze(3).to_broadcast(
                                    [128, 2, NHEADS, HEAD_DIM]),
                            )
                            for j in range(2):
                                nc.tensor.matmul(kvp[:], Bt[:, i + j, :], vst[:, j, :],
                                                 start=(i + j == 0), stop=(i + j == LCH - 1))

                    for n in range(NCH + 2):
                        if n < NCH:
                            inproj_xv(n)
                        if 1 <= n < NCH + 1:
                            conv_v(n - 1)
                        if n >= 2:
                            ssd(n - 2)
                    nc.scalar.copy(kv_sb[DSTATE:2 * DSTATE, :], kvp[:])

            # ---- Phase 3: y, D-skip, LN, gate, XBAR transpose, out_proj ----
            # Quad-batched: 4 L-chunks per iteration; ypr spans 2 PSUM banks.
            with (
                tc.tile_pool(name="pyp", bufs=3, space="PSUM") as pyp,
                tc.tile_pool(name="pout", bufs=2, space="PSUM") as pout,
                tc.tile_pool(name="s6", bufs=3) as s6,
                tc.tile_pool(name="s6g", bufs=2) as s6g,
                tc.tile_pool(name="yzp", bufs=1) as yzp,
            ):
                yz_all = yzp.tile([128, LCH, DIM], BF16, tag="yza")
                slabs = [None] * NCH
                yst = [None] * NCH
                outr = outd.ap().rearrange("(m k) l -> k m l", k=128)

                def emit_a(q):
                    ypr = pyp.tile([128, 4, DIM], F32, tag="ypr")
                    for j in range(4):
                        i = 4 * q + j
                        sl = slice(i * 128, (i + 1) * 128)
                        nc.tensor.matmul(ypr[:, j, :], BC[DSTATE:2 * DSTATE, sl],
                                         kv_sb[DSTATE:2 * DSTATE, :], start=True, stop=True)
                    # D-skip: y += V * D, added straight into PSUM on DVE
                    if generic_d:
                        vtmp = s6.tile([128, 4, DIM], F32, tag="vtmp")
                        nc.vector.tensor_mul(
                            vtmp[:], Vt[:, 4 * q:4 * q + 4, :],
                            dex_bc[:].unsqueeze(1).to_broadcast([128, 4, DIM]))
                        nc.vector.tensor_add(ypr[:], ypr[:], vtmp[:])
                    else:
                        nc.vector.tensor_add(ypr[:], ypr[:], Vt[:, 4 * q:4 * q + 4, :])

                    st = s6.tile([128, 4, 6], F32, tag="st")
                    for j in range(4):
                        nc.vector.bn_stats(st[:, j, :], ypr[:, j, :])
                    yst[q] = (ypr, st)

                def emit_b(q):
                    ypr, st = yst[q]
                    mv = s6.tile([128, 4, 2], F32, tag="mv")
                    for j in range(4):
                        nc.vector.bn_aggr(mv[:, j, :], st[:, j, :])
                    rstd = s6.tile([128, 4], F32, tag="rstd")
                    nc.scalar.activation(rstd[:], mv[:, :, 1], AF.Abs_reciprocal_sqrt,
                                         bias=eps_t[:])
                    negm = s6.tile([128, 4], F32, tag="negm")
                    nc.gpsimd.tensor_scalar_mul(negm[:], mv[:, :, 0], -1.0)
                    nmr = s6.tile([128, 4], F32, tag="nmr")
                    nc.gpsimd.tensor_mul(nmr[:], negm[:], rstd[:])
                    yn = s6.tile([128, 4, DIM], F32, tag="yn")
                    for j in range(4):
                        nc.scalar.activation(yn[:, j, :], ypr[:, j, :], AF.Identity,
                                             bias=nmr[:, j:j + 1], scale=rstd[:, j:j + 1])
                    if has_lnw:
                        nc.vector.tensor_mul(
                            yn[:], yn[:],
                            lnw_bc[:].unsqueeze(1).to_broadcast([128, 4, DIM]))
                    if has_lnb:
                        nc.vector.tensor_add(
                            yn[:], yn[:],
                            lnb_bc[:].unsqueeze(1).to_broadcast([128, 4, DIM]))
                    # gate split between DVE and Pool to halve per-quad latency
                    nc.vector.tensor_mul(yz_all[:, 4 * q:4 * q + 2, :], yn[:, 0:2, :],
                                         zt[:, 4 * q:4 * q + 2, :])
                    nc.gpsimd.tensor_mul(yz_all[:, 4 * q + 2:4 * q + 4, :], yn[:, 2:4, :],
                                         zt[:, 4 * q + 2:4 * q + 4, :])

                def emit_tr(q):
                    # 4 chunks (512 l) per XBAR call; slab [cc, jc, g, l]
                    slab = s6g.tile([128, 4, 2, 128], BF16, tag="yzt")
                    nc.sync.dma_start_transpose(
                        slab[:],
                        yz_all[:, 4 * q:4 * q + 4, :].rearrange("a b c -> a (b c)"))
                    slabs[q] = slab

                def emit_out(q):
                    slab = slabs[q]
                    ob = s6g.tile([128, 2, 512], F32, tag="ob")
                    for mo in range(2):
                        po = pout.tile([128, 512], F32)
                        for k in range(2):
                            nc.tensor.matmul(po[:], wo[:, k, mo * 128:(mo + 1) * 128],
                                             slab[:, :, k, :], start=(k == 0), stop=(k == 1))
                        if mo == 0:
                            nc.vector.tensor_copy(ob[:, mo, :], po[:])
                        else:
                            nc.scalar.copy(ob[:, mo, :], po[:])
                    eng = nc.sync if q % 2 == 0 else nc.scalar
                    eng.dma_start(outr[:, :, q * 512:(q + 1) * 512], ob[:])

                for q in range(NCH + 3):
                    if q < NCH:
                        emit_a(q)
                    if 1 <= q < NCH + 1:
                        emit_b(q - 1)
                    if 2 <= q < NCH + 2:
                        emit_tr(q - 2)
                    if q >= 3:
                        emit_out(q - 3)


_CACHE = {}


def _prep(W_in, conv_w, conv_b, A_log, dt_bias, D, ln_w, ln_b, W_out):
    W_in = np.asarray(W_in, np.float32)
    conv_w = np.asarray(conv_w, np.float32)
    conv_b = np.asarray(conv_b, np.float32)
    A_log = np.asarray(A_log, np.float32)
    dt_bias = np.asarray(dt_bias, np.float32)
    D = np.asarray(D, np.float32)
    ln_w = np.asarray(ln_w, np.float32)
    ln_b = np.asarray(ln_b, np.float32)
    W_out = np.asarray(W_out, np.float32)

    WinT = np.ascontiguousarray(W_in.T)                       # [256, 648]
    w_xbcdt = np.ascontiguousarray(WinT[:, DIM:]).astype(BF)  # [256, 392]
    w_z = np.ascontiguousarray(WinT[:, :DIM]).astype(BF)      # [256, 256]
    w_out_t = np.ascontiguousarray(W_out.T).astype(BF)        # [256, 256]

    cd = np.zeros((128, 27, 128), np.float32)
    idx = np.arange(128)
    for g in range(3):
        for t in range(9):
            cd[idx, g * 9 + t, idx] = conv_w[g * 128 + idx, 0, t // 3, t % 3]

    cbm = np.zeros((128, 3), np.float32)
    for g in range(3):
        cbm[:, g] = conv_b[g * 128:(g + 1) * 128]

    flags = {
        "generic_d": not np.allclose(D, 1.0),
        "has_lnw": not np.allclose(ln_w, 1.0),
        "has_lnb": not np.allclose(ln_b, 0.0),
    }
    shared = dict(
        w_xbcdt=w_xbcdt, w_z=w_z, w_out=w_out_t, convdiag=cd.astype(BF),
        dtbias=dt_bias.reshape(NHEADS, 1).astype(np.float32),
        expa=np.exp(A_log).reshape(NHEADS, 1).astype(np.float32),
        convb=cbm, dexpand=np.repeat(D, HEAD_DIM).reshape(1, DIM).astype(np.float32),
        lnwv=ln_w, lnbv=ln_b,
    )
    return flags, shared


def _get_nc(flags, reps=1):
    key = (tuple(sorted(flags.items())), reps)
    if key not in _CACHE:
        _CACHE[key] = _build(flags, reps)
    return _CACHE[key]


def kernel(x, W_in, conv_w, conv_b, A_log, dt_bias, D, ln_w, ln_b, W_out,
           _trace=False):
    x = np.asarray(x, np.float32)
    flags, shared = _prep(W_in, conv_w, conv_b, A_log, dt_bias, D, ln_w, ln_b, W_out)
    nc = _get_nc(flags)
    xb = x.reshape(B_, DIM, L).astype(BF)
    in_maps = [dict(xb=np.ascontiguousarray(xb[b]), **shared) for b in range(B_)]
    res = run_bass_kernel_spmd(nc, in_maps, core_ids=list(range(NCORES)), trace=_trace)
    out = np.stack([res.results[b]["outb"] for b in range(B_)])
    out = out.reshape(B_, DIM, H_, W_)
    if _trace:
        return out, res
    return out
